# revision 3
# baseline (speedup 1.0000x reference)
"""Trainium2 Bass kernel for nn_LstmConv (GNN message passing + LSTMCell).

Sharding: dst nodes load-balanced across 8 cores (permuted into 49 tiles of
128 slots per core). Per core, edge features are fetched with few BIG
InstDMAGatherAnt gathers (bf16 rows, int16 indices against a half-split
feature table, ~2 gathers per 2-tile group) to amortize the ~1us SWDGE
fixed cost; the segment sum is a chain of bf16 one-hot matmuls into PSUM,
and the LSTMCell runs as two PE matmuls + ACT/DVE epilogue per tile.
Output is written transposed and reassembled on the host.
"""

import sys, os

sys.path.insert(0, "/opt/trn_rl_repo")
sys.path.insert(0, os.path.dirname(os.path.abspath(__file__)))

import numpy as np
from ml_dtypes import bfloat16

N_NODES = 50000
N_EDGES = 800000
H = 128
MSG = 64
P = 128
NCORES = 8
TILES = 49
SLOTS = TILES * P          # 6272 per core
HALF = 25000               # feature table split point (int16 index range)
GT = 2                     # tiles per gather group
MAXCH = 8                  # max 128-idx chunks per dma_gather (SWDGE ring cap)

LAST_EXEC_NS = None
TRACE = False


def _wrap16(idx, width):
    """Pack linear index list into [128, width] (idx j at [j%16, j//16],
    replicated across the 8 gpsimd cores)."""
    out = np.zeros((P, width), np.uint16)
    n = len(idx)
    cols = (n + 15) // 16
    blk = np.zeros((16, width), np.uint16)
    flat = np.zeros(cols * 16, np.uint16)
    flat[:n] = idx
    blk[:, :cols] = flat.reshape(cols, 16).T
    out[:] = np.tile(blk, (8, 1))
    return out


def _host_prep(feat, src0, dst0, src1, dst1, W_ih, W_hh, b_ih, b_hh):
    deg0 = np.bincount(dst0, minlength=N_NODES)
    deg1 = np.bincount(dst1, minlength=N_NODES)
    w = deg0 + deg1

    # snake-assign nodes (sorted by degree desc) into 392 tiles of <=128
    n_tiles_g = NCORES * TILES
    order = np.argsort(-w, kind="stable")
    tile_of_node = np.empty(N_NODES, np.int32)
    pos_in_tile = np.empty(N_NODES, np.int32)
    tcnt = np.zeros(n_tiles_g, np.int32)
    idx = 0
    fwd = True
    while idx < N_NODES:
        rng = range(n_tiles_g) if fwd else range(n_tiles_g - 1, -1, -1)
        for t in rng:
            if idx >= N_NODES:
                break
            if tcnt[t] < P:
                tile_of_node[order[idx]] = t
                pos_in_tile[order[idx]] = tcnt[t]
                tcnt[t] += 1
                idx += 1
        fwd = not fwd

    # balance tiles over cores by weight: snake over tiles sorted by weight
    tile_w = np.zeros(n_tiles_g, np.int64)
    np.add.at(tile_w, tile_of_node, w)
    torder = np.argsort(-tile_w, kind="stable")
    core_of_tile = np.empty(n_tiles_g, np.int32)
    tl_of_tile = np.empty(n_tiles_g, np.int32)
    k = 0
    fwd = True
    for rnd in range(TILES):
        cr = range(NCORES) if fwd else range(NCORES - 1, -1, -1)
        for c in cr:
            core_of_tile[torder[k]] = c
            tl_of_tile[torder[k]] = rnd
            k += 1
        fwd = not fwd

    core_of_node = core_of_tile[tile_of_node]
    slot_of_node = tl_of_tile[tile_of_node] * P + pos_in_tile  # slot within core

    # node_of_slot per core (-1 = ghost)
    node_of_slot = -np.ones((NCORES, SLOTS), np.int64)
    node_of_slot[core_of_node, slot_of_node] = np.arange(N_NODES)

    # per-node combined scales a_e = 1/max(cnt_e,1) * 1/max(has0+has1,1)
    has0 = (deg0 > 0).astype(np.float32)
    has1 = (deg1 > 0).astype(np.float32)
    invc = 1.0 / np.maximum(has0 + has1, 1.0)
    a0 = invc / np.maximum(deg0, 1.0)
    a1 = invc / np.maximum(deg1, 1.0)

    # per-core (tile, half) edge groups (both etypes merged; per-edge scale
    # a_e[dst] is folded into the one-hot matrix values), sorted by slot
    src_all = np.concatenate([src0, src1])
    dst_all = np.concatenate([dst0, dst1])
    a_all = np.concatenate([a0[dst0], a1[dst1]]).astype(np.float64)
    groups = {}   # (core, tl, half) -> (srcs_rel, slot_pos, a_edge)
    c = core_of_node[dst_all]
    s = slot_of_node[dst_all]
    hb = (src_all >= HALF).astype(np.int64)
    key = (((c * TILES + (s // P)) * 2 + hb) * P + (s % P)).astype(np.int64)
    o = np.argsort(key, kind="stable")
    src_s, c_s, s_s, hb_s, a_s = src_all[o], c[o], s[o], hb[o], a_all[o]
    gkey = (c_s * TILES + s_s // P) * 2 + hb_s
    bounds = np.searchsorted(gkey, np.arange(NCORES * TILES * 2 + 1))
    for g in range(NCORES * TILES * 2):
        lo, hi = bounds[g], bounds[g + 1]
        cc, rem = divmod(g, TILES * 2)
        tl, hh = divmod(rem, 2)
        groups[(cc, tl, hh)] = (
            src_s[lo:hi] - HALF * hh,
            (s_s[lo:hi] % P).astype(np.float64),
            a_s[lo:hi])

    # common chunk counts per (tl, half): max over cores
    K_th = np.zeros((TILES, 2), np.int32)
    for tl in range(TILES):
        for hh in range(2):
            m = max(len(groups[(c, tl, hh)][0]) for c in range(NCORES))
            K_th[tl, hh] = (m + 127) // 128

    # chunk layout: per gather group g of GT tiles:
    #   [tiles' (e0,h0) (e1,h0) chunks | tiles' (e0,h1) (e1,h1) chunks]
    # meta is identical across cores (shapes/counts only).
    n_groups = (TILES + GT - 1) // GT
    chunk_meta = []        # (tl, half) per global chunk position
    tile_cols = [[] for _ in range(TILES)]
    gathers = []           # (half, chunk_lo, nch) per gather, in group order
    group_spans = []       # (chunk_lo, nch_total) per group (for hot/is_equal)
    for g in range(n_groups):
        tls = range(g * GT, min((g + 1) * GT, TILES))
        g_lo = len(chunk_meta)
        for hh in range(2):
            h_lo = len(chunk_meta)
            for tl in tls:
                for _ in range(K_th[tl, hh]):
                    tile_cols[tl].append(len(chunk_meta))
                    chunk_meta.append((tl, hh))
            # split the half-span into gathers of <= MAXCH chunks
            nch_h = len(chunk_meta) - h_lo
            s = h_lo
            while nch_h > 0:
                take = min(nch_h, MAXCH)
                gathers.append((hh, s, take))
                s += take
                nch_h -= take
        group_spans.append((g_lo, len(chunk_meta) - g_lo))
    CT2 = len(chunk_meta)

    # per-core data arrays in chunk order
    per_core = []
    for c in range(NCORES):
        idx_flat = np.zeros(CT2 * P, np.uint16)
        doff = np.full((P, CT2), 255.0, np.float32)
        scb = np.zeros((P, CT2), np.float32)
        ch = 0
        while ch < CT2:
            tl, hh = chunk_meta[ch]
            K = K_th[tl, hh]
            srcs, pps, aes = groups[(c, tl, hh)]
            n = len(srcs)
            a = np.zeros(K * P, np.uint16)
            a[:n] = srcs
            d = np.full(K * P, 255.0, np.float32)
            d[:n] = pps
            av = np.zeros(K * P, np.float32)
            av[:n] = aes
            idx_flat[ch * P : (ch + K) * P] = a
            doff[:, ch : ch + K] = d.reshape(K, P).T
            scb[:, ch : ch + K] = av.reshape(K, P).T
            ch += K
        # wrap16 index layout per gather span
        gw_parts = []
        for hh, lo, nch in gathers:
            if nch == 0:
                continue
            gw_parts.append(
                _wrap16(idx_flat[lo * P : (lo + nch) * P], nch * 8))
        gidx16 = np.concatenate(gw_parts, axis=1).view(np.int16).copy()

        # local node features, transposed, bf16
        sl = node_of_slot[c]
        floc = np.zeros((SLOTS, H), np.float32)
        floc[sl >= 0] = feat[sl[sl >= 0]]
        per_core.append(dict(gidx16=gidx16, doff=doff.astype(bfloat16),
                             scb=scb.astype(bfloat16),
                             featloc=floc.T.astype(bfloat16).copy()))

    # half-split feature tables, bf16
    featA = feat[:HALF].astype(bfloat16)
    featB = np.zeros((HALF, H), np.float32)
    featB[: N_NODES - HALF] = feat[HALF:]
    featB = featB.astype(bfloat16)

    # gate order [i, f, g, o] (PyTorch native)
    wih = W_ih.T.astype(bfloat16).copy()              # [128, 256]
    whh = W_hh.T.astype(bfloat16).copy()              # [64, 256]
    bt = (b_ih + b_hh).astype(np.float32)
    biasT = np.stack([bt[:128], bt[128:]], axis=1).copy()  # [128, 2]
    iota = np.tile(np.arange(P, dtype=np.float32)[None, :], (P, 1)).astype(bfloat16)

    shared = dict(featA=featA, featB=featB, wih=wih, whh=whh, biasT=biasT,
                  iota=iota)
    meta = dict(CT2=CT2, gathers=gathers, group_spans=group_spans,
                tile_cols=tile_cols, n_groups=n_groups,
                GW=sum(nch * 8 for _, _, nch in gathers))
    return per_core, shared, node_of_slot, meta


_WS = [0]


def _split_multi_waits(nc, mybir, max_waits=1):
    """This container's walrus rejects >1 sync wait per instruction; split
    extra waits onto single-wait NoOps placed just before the instruction."""
    for fn in nc.m.functions:
        for bb in fn.blocks:
            new = []
            for ins in bb.instructions:
                si = ins.sync_info
                if si is not None and len(si.on_wait) > max_waits:
                    waits = list(si.on_wait)
                    for w in waits[:-max_waits]:
                        _WS[0] += 1
                        nop = mybir.InstNoOp(
                            name=f"I-waitsplit-{_WS[0]}", ins=[], outs=[]
                        )
                        nop.engine = ins.engine
                        nop.sync_info = mybir.SyncInfo(on_wait=[w], on_update=[])
                        new.append(nop)
                    si.on_wait = waits[-max_waits:]
                new.append(ins)
            bb.instructions[:] = new


def _build_nc(meta):
    from concourse import bass, mybir, tile, library_config
    from concourse.masks import make_identity

    f32, bf16, i16 = mybir.dt.float32, mybir.dt.bfloat16, mybir.dt.int16
    CT2 = meta["CT2"]
    gathers = meta["gathers"]
    group_spans = meta["group_spans"]
    tile_cols = meta["tile_cols"]
    n_groups = meta["n_groups"]
    GW = meta["GW"]

    nc = bass.Bass(num_swdge_queues=4)
    featA_d = nc.declare_dram_parameter("featA", [HALF, H], bf16, isOutput=False)
    featB_d = nc.declare_dram_parameter("featB", [HALF, H], bf16, isOutput=False)
    gidx_d = nc.declare_dram_parameter("gidx16", [P, GW], i16, isOutput=False)
    doff_d = nc.declare_dram_parameter("doff", [P, CT2], bf16, isOutput=False)
    scb_d = nc.declare_dram_parameter("scb", [P, CT2], bf16, isOutput=False)
    wih_d = nc.declare_dram_parameter("wih", [P, 256], bf16, isOutput=False)
    whh_d = nc.declare_dram_parameter("whh", [64, 256], bf16, isOutput=False)
    bias_d = nc.declare_dram_parameter("biasT", [P, 2], f32, isOutput=False)
    iota_d = nc.declare_dram_parameter("iota", [P, P], bf16, isOutput=False)
    floc_d = nc.declare_dram_parameter("featloc", [P, SLOTS], bf16, isOutput=False)
    outT_d = nc.declare_dram_parameter("outT", [P, SLOTS], f32, isOutput=True)

    KBUF = max(n for _, n in group_spans)

    with tile.TileContext(nc) as tc:
        with (
            tc.tile_pool(name="const", bufs=1) as cp,
            tc.tile_pool(name="gb", bufs=2) as gbp,
            tc.tile_pool(name="hot", bufs=2) as hp,
            tc.tile_pool(name="ep", bufs=2) as ep,
            tc.tile_pool(name="psm", bufs=2, space="PSUM") as psm,
            tc.tile_pool(name="psg", bufs=1, space="PSUM") as psgp,
        ):
            nc.gpsimd.load_library(library_config.mlp)
            niregs = {}
            for hh, lo, nch in gathers:
                if nch and nch * P not in niregs:
                    niregs[nch * P] = nc.gpsimd.to_reg(nch * P)
            gidx = cp.tile([P, GW], i16)
            nc.sync.dma_start(out=gidx[:], in_=gidx_d[:])
            doff = cp.tile([P, CT2], bf16)
            nc.sync.dma_start(out=doff[:], in_=doff_d[:])
            scb = cp.tile([P, CT2], bf16)
            nc.sync.dma_start(out=scb[:], in_=scb_d[:])
            wih = cp.tile([P, 256], bf16)
            nc.sync.dma_start(out=wih[:], in_=wih_d[:])
            whh = cp.tile([64, 256], bf16)
            nc.sync.dma_start(out=whh[:], in_=whh_d[:])
            bias = cp.tile([P, 2], f32)
            nc.sync.dma_start(out=bias[:], in_=bias_d[:])
            iota = cp.tile([P, P], bf16)
            nc.sync.dma_start(out=iota[:], in_=iota_d[:])
            featloc = cp.tile([P, SLOTS], bf16)
            nc.sync.dma_start(out=featloc[:], in_=floc_d[:])

            gi = 0      # gather index
            icol = 0    # idx16 column cursor
            for g in range(n_groups):
                g_lo, g_nch = group_spans[g]
                gb = gbp.tile([P, KBUF, P], bf16, tag="gb")
                while gi < len(gathers) and gathers[gi][1] < g_lo + g_nch:
                    hh, lo, nch = gathers[gi]
                    gi += 1
                    if nch == 0:
                        continue
                    nc.gpsimd.dma_gather(
                        out_ap=gb[:, lo - g_lo : lo - g_lo + nch, :],
                        in_ap=(featA_d if hh == 0 else featB_d)[:],
                        idxs_ap=gidx[:, icol : icol + nch * 8],
                        num_idxs=nch * P,
                        num_idxs_reg=niregs[nch * P],
                        elem_size=H,
                        queue_num=gi % 4,
                    )
                    icol += nch * 8
                hot = hp.tile([P, KBUF * P], bf16, tag="hot")
                nc.vector.tensor_tensor(
                    out=hot[:, : g_nch * P],
                    in0=doff[:, g_lo : g_lo + g_nch].to_broadcast([P, g_nch, P]),
                    in1=iota[:, None, :].to_broadcast([P, g_nch, P]),
                    op=mybir.AluOpType.is_equal,
                )
                nc.vector.tensor_tensor(
                    out=hot[:, : g_nch * P],
                    in0=hot[:, : g_nch * P],
                    in1=scb[:, g_lo : g_lo + g_nch].to_broadcast([P, g_nch, P]),
                    op=mybir.AluOpType.mult,
                )
                for tl in range(g * GT, min((g + 1) * GT, TILES)):
                    cols = tile_cols[tl]
                    # swapped operands: pm = gb^T @ hot = rst^T [h, slot]
                    pm = psm.tile([P, P], f32, tag="m0")
                    for j, ch in enumerate(cols):
                        k = ch - g_lo
                        nc.tensor.matmul(
                            out=pm[:],
                            lhsT=gb[:, k, :],
                            rhs=hot[:, k * P : (k + 1) * P],
                            start=(j == 0), stop=(j == len(cols) - 1),
                        )
                    # pm IS rst^T: rows 0:64 = G_t^T (gates rhs), 64:128 = R_t^T
                    rstb = ep.tile([64, P], bf16, tag="rstb")
                    nc.vector.tensor_copy(out=rstb[:], in_=pm[0:64, :])
                    # gates
                    pg = []
                    for half in range(2):
                        g_ps = psgp.tile([P, P], f32, tag=f"pg{half}")
                        nc.tensor.matmul(
                            out=g_ps[:], lhsT=wih[:, half * P : (half + 1) * P],
                            rhs=featloc[:, tl * P : (tl + 1) * P],
                            start=True, stop=False,
                        )
                        nc.tensor.matmul(
                            out=g_ps[:], lhsT=whh[:, half * P : (half + 1) * P],
                            rhs=rstb[:], start=False, stop=True,
                        )
                        pg.append(g_ps)
                    # gates halves: pg0 = [i; f], pg1 = [g; o]
                    sif = ep.tile([P, P], f32, tag="sif")
                    nc.scalar.activation(
                        out=sif[:], in_=pg[0][:],
                        func=mybir.ActivationFunctionType.Sigmoid, bias=bias[:, 0:1],
                    )
                    sog = ep.tile([P, P], f32, tag="sog")
                    nc.scalar.activation(
                        out=sog[0:64, :], in_=pg[1][0:64, :],
                        func=mybir.ActivationFunctionType.Tanh, bias=bias[0:64, 1:2],
                    )
                    nc.scalar.activation(
                        out=sog[64:128, :], in_=pg[1][64:128, :],
                        func=mybir.ActivationFunctionType.Sigmoid, bias=bias[64:128, 1:2],
                    )
                    outsb = ep.tile([P, P], f32, tag="outsb")
                    t2 = ep.tile([64, P], f32, tag="t2")
                    tt = ep.tile([P, P], f32, tag="tt")
                    nc.vector.tensor_tensor(
                        out=t2[:], in0=sif[0:64, :], in1=sog[0:64, :],
                        op=mybir.AluOpType.mult,
                    )
                    nc.scalar.activation(
                        out=tt[64:128, :], in_=t2[:],
                        func=mybir.ActivationFunctionType.Copy,
                    )
                    nc.vector.tensor_tensor(
                        out=outsb[64:128, :], in0=sif[64:128, :], in1=pm[64:128, :],
                        op=mybir.AluOpType.mult,
                    )
                    nc.vector.tensor_tensor(
                        out=outsb[64:128, :], in0=outsb[64:128, :], in1=tt[64:128, :],
                        op=mybir.AluOpType.add,
                    )
                    nc.scalar.activation(
                        out=tt[64:128, :], in_=outsb[64:128, :],
                        func=mybir.ActivationFunctionType.Tanh,
                    )
                    nc.vector.tensor_tensor(
                        out=tt[64:128, :], in0=sog[64:128, :], in1=tt[64:128, :],
                        op=mybir.AluOpType.mult,
                    )
                    nc.scalar.activation(
                        out=outsb[0:64, :], in_=tt[64:128, :],
                        func=mybir.ActivationFunctionType.Copy,
                    )
                    nc.sync.dma_start(
                        out=outT_d[:, tl * P : (tl + 1) * P], in_=outsb[:]
                    )
    from concourse import mybir as _mb
    _mb.codegen_inst_isa_subclasses(nc)
    _split_multi_waits(nc, mybir)
    return nc


def kernel(feat, src0, dst0, src1, dst1, W_ih, W_hh, b_ih, b_hh):
    global LAST_EXEC_NS
    feat = np.asarray(feat, np.float32)
    src0 = np.asarray(src0, np.int64); dst0 = np.asarray(dst0, np.int64)
    src1 = np.asarray(src1, np.int64); dst1 = np.asarray(dst1, np.int64)
    per_core, shared, node_of_slot, meta = _host_prep(
        feat, src0, dst0, src1, dst1,
        np.asarray(W_ih, np.float32), np.asarray(W_hh, np.float32),
        np.asarray(b_ih, np.float32), np.asarray(b_hh, np.float32),
    )
    nc = _build_nc(meta)
    in_maps = [{**shared, **pc} for pc in per_core]
    from concourse.bass_utils import run_bass_kernel_spmd
    res = run_bass_kernel_spmd(
        nc, in_maps, list(range(NCORES)), trace=TRACE,
        tmpdir="/tmp/trace_out" if TRACE else None,
    )
    LAST_EXEC_NS = res.exec_time_ns
    out = np.zeros((N_NODES, H), np.float32)
    for c in range(NCORES):
        oT = res.results[c]["outT"]          # [128, SLOTS]
        valid = node_of_slot[c] >= 0
        nodes = node_of_slot[c][valid]
        blk = oT.T[valid]                    # [n, 128]: cols 0:64=h1, 64:128=c1
        out[nodes] = blk
    return out



# revision 4
# speedup vs baseline: 1.0832x; 1.0832x over previous
"""Trainium2 Bass kernel for nn_LstmConv (GNN message passing + LSTMCell), v2.

Architecture (per core, dst-node sharding):
- Edges sorted into two continuous position streams (one per feature-table
  half), ordered by (tile, etype, slot); cells (tile, etype, half) padded to
  16-granular sizes common across cores (SPMD uniformity), ~4% pad.
- Streams gathered with 1024-index SWDGE dma_gather windows, 4 queues, deep
  buffer pools so the DMA rings never starve (descriptor-rate-bound at
  ~2.3ns/desc).
- One-hot matrices are pure 0/1, built by a single DVE is_equal pass per
  window (doff column per piece; a piece = chunk x cell intersection).
- Per (tile, etype) PSUM accumulates gb^T @ hot pieces; epilogue applies
  per-slot scales a_e[slot] (folding 1/deg and 1/n_active) and computes the
  LSTMCell exactly as the baseline.
"""

import sys, os

sys.path.insert(0, "/opt/trn_rl_repo")
sys.path.insert(0, os.path.dirname(os.path.abspath(__file__)))

import numpy as np
from ml_dtypes import bfloat16

N_NODES = 50000
N_EDGES = 800000
H = 128
MSG = 64
P = 128
NCORES = 8
TILES = 49
SLOTS = TILES * P          # 6272 per core
HALF = 25000
WIN = 1024                 # indices per gather window
CELL_GRAN = 16

LAST_EXEC_NS = None
TRACE = False


def _wrap16(idx):
    """[1024] -> [128, 64] wrap: idx j at [j%16, j//16], replicated x8."""
    n = len(idx)
    cols = (n + 15) // 16
    blk = np.zeros((16, cols), np.uint16)
    flat = np.zeros(cols * 16, np.uint16)
    flat[:n] = idx
    blk[:, :] = flat.reshape(cols, 16).T
    return np.tile(blk, (8, 1))


def _host_prep(feat, src0, dst0, src1, dst1, W_ih, W_hh, b_ih, b_hh):
    deg0 = np.bincount(dst0, minlength=N_NODES)
    deg1 = np.bincount(dst1, minlength=N_NODES)
    w = deg0 + deg1

    # snake-assign nodes (sorted by degree desc) into 392 tiles of <=128
    n_tiles_g = NCORES * TILES
    order = np.argsort(-w, kind="stable")
    tile_of_node = np.empty(N_NODES, np.int32)
    pos_in_tile = np.empty(N_NODES, np.int32)
    tcnt = np.zeros(n_tiles_g, np.int32)
    idx = 0
    fwd = True
    while idx < N_NODES:
        rng = range(n_tiles_g) if fwd else range(n_tiles_g - 1, -1, -1)
        for t in rng:
            if idx >= N_NODES:
                break
            if tcnt[t] < P:
                tile_of_node[order[idx]] = t
                pos_in_tile[order[idx]] = tcnt[t]
                tcnt[t] += 1
                idx += 1
        fwd = not fwd

    # balance tiles over cores by weight
    tile_w = np.zeros(n_tiles_g, np.int64)
    np.add.at(tile_w, tile_of_node, w)
    torder = np.argsort(-tile_w, kind="stable")
    core_of_tile = np.empty(n_tiles_g, np.int32)
    tl_of_tile = np.empty(n_tiles_g, np.int32)
    k = 0
    fwd = True
    for rnd in range(TILES):
        cr = range(NCORES) if fwd else range(NCORES - 1, -1, -1)
        for c in cr:
            core_of_tile[torder[k]] = c
            tl_of_tile[torder[k]] = rnd
            k += 1
        fwd = not fwd

    core_of_node = core_of_tile[tile_of_node]
    slot_of_node = tl_of_tile[tile_of_node] * P + pos_in_tile

    node_of_slot = -np.ones((NCORES, SLOTS), np.int64)
    node_of_slot[core_of_node, slot_of_node] = np.arange(N_NODES)

    # per-node scales: a_e = 1/max(deg_e,1) / max(has0+has1,1)
    has0 = (deg0 > 0).astype(np.float32)
    has1 = (deg1 > 0).astype(np.float32)
    invc = 1.0 / np.maximum(has0 + has1, 1.0)
    a0 = (invc / np.maximum(deg0, 1.0)).astype(np.float32)
    a1 = (invc / np.maximum(deg1, 1.0)).astype(np.float32)

    # edge table
    src_all = np.concatenate([src0, src1])
    dst_all = np.concatenate([dst0, dst1])
    et_all = np.concatenate([np.zeros(len(src0), np.int64),
                             np.ones(len(src1), np.int64)])
    c_all = core_of_node[dst_all]
    s_all = slot_of_node[dst_all]
    t_all = s_all // P
    sp_all = s_all % P
    h_all = (src_all >= HALF).astype(np.int64)
    srel_all = src_all - HALF * h_all

    # cell counts [core, tile, etype, half]
    cell_id = ((t_all * 2 + et_all) * 2 + h_all)
    cnt = np.zeros((NCORES, TILES * 4), np.int64)
    np.add.at(cnt, (c_all, cell_id), 1)
    CS = cnt.max(axis=0)                     # [TILES*4]
    CS = np.maximum((CS + CELL_GRAN - 1) // CELL_GRAN * CELL_GRAN, CELL_GRAN)
    CS = CS.reshape(TILES, 2, 2)             # [tile, etype, half]

    # stream layout per half: cells in (tile, etype) order
    stream_len = [0, 0]
    cell_off = np.zeros((TILES, 2, 2), np.int64)
    for t in range(TILES):
        for e in range(2):
            for h in range(2):
                cell_off[t, e, h] = stream_len[h]
                stream_len[h] += CS[t, e, h]
    # pad stream lengths to WIN multiple (extend last cell)
    for h in range(2):
        L = (stream_len[h] + WIN - 1) // WIN * WIN
        CS[TILES - 1, 1, h] += L - stream_len[h]
        stream_len[h] = L
    LA, LB = stream_len

    # piece table (identical across cores): walk each half-stream
    # piece = (half, window, chunk_in_win, tile, etype, pos_lo, pos_hi)
    pieces = []            # in per-half stream order
    for h in range(2):
        bounds = []        # (stream_pos_end, tile, etype)
        for t in range(TILES):
            for e in range(2):
                bounds.append((cell_off[t, e, h] + CS[t, e, h], t, e))
        L = stream_len[h]
        ci = 0
        pos = 0
        while pos < L:
            chunk_end = pos - pos % P + P
            cell_end, t, e = bounds[ci]
            hi = min(chunk_end, cell_end)
            pieces.append((h, pos // WIN, (pos % WIN) // P, t, e, pos, hi))
            if hi == cell_end and ci + 1 < len(bounds):
                ci += 1
            pos = hi
    NPIECES = len(pieces)

    # program-order interleave of windows: merge by tile progress so the two
    # streams stay within ~1 window of each other in tile space (bounds the
    # number of live PSUM tiles)
    nwin = [LA // WIN, LB // WIN]
    first_tile = {}
    for (h, wi, ck, t, e, lo, hi) in pieces:
        if (h, wi) not in first_tile:
            first_tile[(h, wi)] = t
    win_order = sorted(
        [(h, w) for h in range(2) for w in range(nwin[h])],
        key=lambda hw: (first_tile.get(hw, TILES), hw[1], hw[0]),
    )

    # pieces grouped by (half, window)
    pieces_by_win = {}
    for pi, pc in enumerate(pieces):
        pieces_by_win.setdefault((pc[0], pc[1]), []).append(pi)

    # start/stop flags per (tile, etype) in program order
    order_of_win = {hw: i for i, hw in enumerate(win_order)}
    first_piece = {}
    last_piece = {}
    for pi, (h, wi, ck, t, e, lo, hi) in enumerate(pieces):
        key = (t, e)
        rank = (order_of_win[(h, wi)], pi)
        if key not in first_piece or rank < first_piece[key][0]:
            first_piece[key] = (rank, pi)
        if key not in last_piece or rank > last_piece[key][0]:
            last_piece[key] = (rank, pi)
    start_of = {v[1] for v in first_piece.values()}
    stop_of = {v[1]: k for k, v in last_piece.items()}

    # per-core stream fill
    per_core = []
    for c in range(NCORES):
        m = c_all == c
        key = (h_all[m], t_all[m], et_all[m], sp_all[m])
        o = np.lexsort((key[3], key[2], key[1], key[0]))
        hs, ts, es, sps, srs = (h_all[m][o], t_all[m][o], et_all[m][o],
                                sp_all[m][o], srel_all[m][o])
        sidx = [np.zeros(LA, np.uint16), np.zeros(LB, np.uint16)]
        sdoff = [np.full(LA, 255.0, np.float32), np.full(LB, 255.0, np.float32)]
        # fill cells (edges sorted by (h, t, e, slot))
        cellk = (hs * TILES + ts) * 2 + es
        ccnt = np.bincount(cellk, minlength=TILES * 4)
        cbnd = np.concatenate([[0], np.cumsum(ccnt)])
        for h in range(2):
            for t in range(TILES):
                for e in range(2):
                    kk = (h * TILES + t) * 2 + e
                    lo2, hi2 = cbnd[kk], cbnd[kk + 1]
                    n = hi2 - lo2
                    off = cell_off[t, e, h]
                    assert n <= CS[t, e, h], (c, t, e, h, n, CS[t, e, h])
                    sidx[h][off:off + n] = srs[lo2:hi2]
                    sdoff[h][off:off + n] = sps[lo2:hi2]
        # gidx: wrap16 per window, concatenated in window ISSUE order so the
        # k-th slice of the gidx DMA covers the k-th issued gathers
        gw = [_wrap16(sidx[h][w * WIN:(w + 1) * WIN]) for (h, w) in win_order]
        gidx16 = np.concatenate(gw, axis=1).view(np.int16).copy()
        # doffP [128, NPIECES]
        doffP = np.full((P, NPIECES), 255.0, np.float32)
        for pi, (h, wi, ck, t, e, lo, hi) in enumerate(pieces):
            v = sdoff[h][lo:hi]
            doffP[lo % P:(lo % P) + (hi - lo), pi] = v
        # local node features, transposed, bf16
        sl = node_of_slot[c]
        floc = np.zeros((SLOTS, H), np.float32)
        floc[sl >= 0] = feat[sl[sl >= 0]]
        # per-slot scale tables, replicated across partitions
        a0s = np.zeros(SLOTS, np.float32)
        a1s = np.zeros(SLOTS, np.float32)
        a0s[sl >= 0] = a0[sl[sl >= 0]]
        a1s[sl >= 0] = a1[sl[sl >= 0]]
        a0rep = np.tile(a0s[None, :], (P, 1)).astype(bfloat16)
        a1rep = np.tile(a1s[None, :], (P, 1)).astype(bfloat16)
        per_core.append(dict(
            gidx16=gidx16, doffP=doffP.astype(bfloat16),
            a0rep=a0rep, a1rep=a1rep,
            featloc=floc.T.astype(bfloat16).copy()))

    featA = feat[:HALF].astype(bfloat16)
    featB = np.zeros((HALF, H), np.float32)
    featB[: N_NODES - HALF] = feat[HALF:]
    featB = featB.astype(bfloat16)

    wih = W_ih.T.astype(bfloat16).copy()              # [128, 256]
    whh = W_hh.T.astype(bfloat16).copy()              # [64, 256]
    bt = (b_ih + b_hh).astype(np.float32)
    biasT = np.stack([bt[:128], bt[128:]], axis=1).copy()  # [128, 2]
    iota = np.tile(np.arange(P, dtype=np.float32)[None, :], (P, 1)).astype(bfloat16)

    shared = dict(featA=featA, featB=featB, wih=wih, whh=whh, biasT=biasT,
                  iota=iota)
    meta = dict(pieces=pieces, pieces_by_win=pieces_by_win,
                win_order=win_order, nwin=nwin, NPIECES=NPIECES,
                start_of=start_of, stop_of=stop_of,
                GW=(nwin[0] + nwin[1]) * (WIN // 16))
    return per_core, shared, node_of_slot, meta


_WS = [0]


def _split_multi_waits(nc, mybir, max_waits=1):
    for fn in nc.m.functions:
        for bb in fn.blocks:
            new = []
            for ins in bb.instructions:
                si = ins.sync_info
                if si is not None and len(si.on_wait) > max_waits:
                    waits = list(si.on_wait)
                    for w in waits[:-max_waits]:
                        _WS[0] += 1
                        nop = mybir.InstNoOp(
                            name=f"I-waitsplit-{_WS[0]}", ins=[], outs=[]
                        )
                        nop.engine = ins.engine
                        nop.sync_info = mybir.SyncInfo(on_wait=[w], on_update=[])
                        new.append(nop)
                    si.on_wait = waits[-max_waits:]
                new.append(ins)
            bb.instructions[:] = new


def _build_nc(meta):
    from concourse import bass, mybir, tile, library_config

    f32, bf16, i16 = mybir.dt.float32, mybir.dt.bfloat16, mybir.dt.int16
    pieces = meta["pieces"]
    pieces_by_win = meta["pieces_by_win"]
    win_order = meta["win_order"]
    nwin = meta["nwin"]
    NPIECES = meta["NPIECES"]
    start_of = meta["start_of"]
    stop_of = meta["stop_of"]
    GW = meta["GW"]

    nc = bass.Bass(num_swdge_queues=4)
    featA_d = nc.declare_dram_parameter("featA", [HALF, H], bf16, isOutput=False)
    featB_d = nc.declare_dram_parameter("featB", [HALF, H], bf16, isOutput=False)
    gidx_d = nc.declare_dram_parameter("gidx16", [P, GW], i16, isOutput=False)
    doff_d = nc.declare_dram_parameter("doffP", [P, NPIECES], bf16, isOutput=False)
    a0_d = nc.declare_dram_parameter("a0rep", [P, SLOTS], bf16, isOutput=False)
    a1_d = nc.declare_dram_parameter("a1rep", [P, SLOTS], bf16, isOutput=False)
    wih_d = nc.declare_dram_parameter("wih", [P, 256], bf16, isOutput=False)
    whh_d = nc.declare_dram_parameter("whh", [64, 256], bf16, isOutput=False)
    bias_d = nc.declare_dram_parameter("biasT", [P, 2], f32, isOutput=False)
    iota_d = nc.declare_dram_parameter("iota", [P, P], bf16, isOutput=False)
    floc_d = nc.declare_dram_parameter("featloc", [P, SLOTS], bf16, isOutput=False)
    outT_d = nc.declare_dram_parameter("outT", [P, SLOTS], f32, isOutput=True)

    # window index base (into gidx cols) per (half, wi): issue order
    gidx_col = {}
    col = 0
    for hw in win_order:
        gidx_col[hw] = col
        col += WIN // 16

    with tile.TileContext(nc) as tc:
        with (
            tc.tile_pool(name="const", bufs=1) as cp,
            tc.tile_pool(name="gba", bufs=10) as gba,
            tc.tile_pool(name="gbb", bufs=10) as gbb,
            tc.tile_pool(name="hot", bufs=6) as hp,
            tc.tile_pool(name="ep", bufs=2) as ep,
            tc.tile_pool(name="psm", bufs=2, space="PSUM") as psm,
            tc.tile_pool(name="psg", bufs=1, space="PSUM") as psgp,
        ):
            nc.gpsimd.load_library(library_config.mlp)
            nireg = nc.gpsimd.to_reg(WIN)
            # gidx in separate slice tiles so the first gathers start
            # almost immediately (each gather depends only on its slice)
            NSL = 8
            sl = ((GW + NSL - 1) // NSL + 63) // 64 * 64
            gidx_sl = []
            for s in range(0, GW, sl):
                e_ = min(s + sl, GW)
                g_t = cp.tile([P, e_ - s], i16, name=f"gidx{s}")
                gidx_sl.append((s, e_, g_t))
                nc.sync.dma_start(out=g_t[:], in_=gidx_d[:, s:e_])
                if s == 0:
                    doff = cp.tile([P, NPIECES], bf16)
                    nc.sync.dma_start(out=doff[:], in_=doff_d[:])
                    iota = cp.tile([P, P], bf16)
                    nc.sync.dma_start(out=iota[:], in_=iota_d[:])

            def gidx_ap(c0, c1):
                for s, e_, g_t in gidx_sl:
                    if c0 >= s and c1 <= e_:
                        return g_t[:, c0 - s:c1 - s]
                raise AssertionError((c0, c1))
            a0rep = cp.tile([P, SLOTS], bf16)
            nc.sync.dma_start(out=a0rep[:], in_=a0_d[:])
            a1rep = cp.tile([P, SLOTS], bf16)
            nc.sync.dma_start(out=a1rep[:], in_=a1_d[:])
            wih = cp.tile([P, 256], bf16)
            nc.sync.dma_start(out=wih[:], in_=wih_d[:])
            whh = cp.tile([64, 256], bf16)
            nc.sync.dma_start(out=whh[:], in_=whh_d[:])
            bias = cp.tile([P, 2], f32)
            nc.sync.dma_start(out=bias[:], in_=bias_d[:])
            featloc = cp.tile([P, SLOTS], bf16)
            nc.sync.dma_start(out=featloc[:], in_=floc_d[:])

            gb_tiles = {}     # (half, wi) -> tile
            hot_tiles = {}    # (half, wi) -> (tile, piece_lo)
            pm_tiles = {}     # (tile, etype) -> psum tile

            def issue_gather(h, wi):
                pool = gba if h == 0 else gbb
                gb = pool.tile([P, WIN // P, P], bf16, tag="gb")
                gb_tiles[(h, wi)] = gb
                nc.gpsimd.dma_gather(
                    out_ap=gb[:],
                    in_ap=(featA_d if h == 0 else featB_d)[:],
                    idxs_ap=gidx_ap(gidx_col[(h, wi)],
                                    gidx_col[(h, wi)] + WIN // 16),
                    num_idxs=WIN,
                    num_idxs_reg=nireg,
                    elem_size=H,
                    queue_num=(wi * 2 + h) % 4,
                )

            PREFETCH = 8
            for k in range(min(PREFETCH, len(win_order))):
                issue_gather(*win_order[k])

            done_tiles = set()
            # count remaining stop flags per tile to trigger epilogue
            stops_needed = {}
            for pi, te in stop_of.items():
                stops_needed.setdefault(te[0], set()).add(te[1])

            def lstm_tile(tl, rstb):
                pg = []
                for half in range(2):
                    g_ps = psgp.tile([P, P], f32, tag=f"pg{half}")
                    nc.tensor.matmul(
                        out=g_ps[:], lhsT=wih[:, half * P:(half + 1) * P],
                        rhs=featloc[:, tl * P:(tl + 1) * P],
                        start=True, stop=False,
                    )
                    nc.tensor.matmul(
                        out=g_ps[:], lhsT=whh[:, half * P:(half + 1) * P],
                        rhs=rstb[0:64, :], start=False, stop=True,
                    )
                    pg.append(g_ps)
                sif = ep.tile([P, P], f32, tag="sif")
                nc.scalar.activation(
                    out=sif[:], in_=pg[0][:],
                    func=mybir.ActivationFunctionType.Sigmoid, bias=bias[:, 0:1],
                )
                sog = ep.tile([P, P], f32, tag="sog")
                nc.scalar.activation(
                    out=sog[0:64, :], in_=pg[1][0:64, :],
                    func=mybir.ActivationFunctionType.Tanh, bias=bias[0:64, 1:2],
                )
                nc.scalar.activation(
                    out=sog[64:128, :], in_=pg[1][64:128, :],
                    func=mybir.ActivationFunctionType.Sigmoid, bias=bias[64:128, 1:2],
                )
                outsb = ep.tile([P, P], f32, tag="outsb")
                t2 = ep.tile([64, P], f32, tag="t2")
                tt = ep.tile([P, P], f32, tag="tt")
                nc.vector.tensor_tensor(
                    out=t2[:], in0=sif[0:64, :], in1=sog[0:64, :],
                    op=mybir.AluOpType.mult,
                )
                nc.scalar.activation(
                    out=tt[64:128, :], in_=t2[:],
                    func=mybir.ActivationFunctionType.Copy,
                )
                nc.vector.tensor_tensor(
                    out=outsb[64:128, :], in0=sif[64:128, :], in1=rstb[64:128, :],
                    op=mybir.AluOpType.mult,
                )
                nc.vector.tensor_tensor(
                    out=outsb[64:128, :], in0=outsb[64:128, :], in1=tt[64:128, :],
                    op=mybir.AluOpType.add,
                )
                nc.scalar.activation(
                    out=tt[64:128, :], in_=outsb[64:128, :],
                    func=mybir.ActivationFunctionType.Tanh,
                )
                nc.vector.tensor_tensor(
                    out=tt[64:128, :], in0=sog[64:128, :], in1=tt[64:128, :],
                    op=mybir.AluOpType.mult,
                )
                nc.scalar.activation(
                    out=outsb[0:64, :], in_=tt[64:128, :],
                    func=mybir.ActivationFunctionType.Copy,
                )
                nc.sync.dma_start(
                    out=outT_d[:, tl * P:(tl + 1) * P], in_=outsb[:]
                )

            for k, (h, wi) in enumerate(win_order):
                if k + PREFETCH < len(win_order):
                    issue_gather(*win_order[k + PREFETCH])
                plist = pieces_by_win.get((h, wi), [])
                if not plist:
                    continue
                p_lo = plist[0]
                npz = len(plist)
                # one is_equal builds all hots of this window
                hot = hp.tile([P, npz, P], bf16, tag="hot")
                nc.vector.tensor_tensor(
                    out=hot[:],
                    in0=doff[:, p_lo:p_lo + npz].to_broadcast([P, npz, P]),
                    in1=iota[:, None, :].to_broadcast([P, npz, P]),
                    op=mybir.AluOpType.is_equal,
                )
                gb = gb_tiles.pop((h, wi))
                for j, pi in enumerate(plist):
                    ph, pwi, ck, t, e, lo, hi = pieces[pi]
                    key = (t, e)
                    if pi in start_of:
                        pm_tiles[key] = psm.tile(
                            [P, P], f32, tag=f"pm{e}", name=f"pm{e}_{t}")
                    nc.tensor.matmul(
                        out=pm_tiles[key][:],
                        lhsT=gb[:, ck, :],
                        rhs=hot[:, j, :],
                        start=(pi in start_of),
                        stop=(pi in stop_of),
                    )
                    if pi in stop_of:
                        te = stop_of[pi]
                        tl = te[0]
                        stops_needed[tl].discard(te[1])
                        if not stops_needed[tl]:
                            # epilogue: rstb = pm0*a0 + pm1*a1 (bf16)
                            pm0 = pm_tiles.pop((tl, 0))
                            pm1 = pm_tiles.pop((tl, 1))
                            tta = ep.tile([P, P], f32, tag="tta")
                            nc.vector.tensor_tensor(
                                out=tta[:], in0=pm0[:],
                                in1=a0rep[:, tl * P:(tl + 1) * P],
                                op=mybir.AluOpType.mult,
                            )
                            ttb = ep.tile([P, P], f32, tag="ttb")
                            nc.vector.tensor_tensor(
                                out=ttb[:], in0=pm1[:],
                                in1=a1rep[:, tl * P:(tl + 1) * P],
                                op=mybir.AluOpType.mult,
                            )
                            rstb = ep.tile([P, P], bf16, tag="rstb")
                            nc.vector.tensor_tensor(
                                out=rstb[:], in0=tta[:], in1=ttb[:],
                                op=mybir.AluOpType.add,
                            )
                            lstm_tile(tl, rstb)

    from concourse import mybir as _mb
    _mb.codegen_inst_isa_subclasses(nc)
    _split_multi_waits(nc, mybir)
    return nc


def kernel(feat, src0, dst0, src1, dst1, W_ih, W_hh, b_ih, b_hh):
    global LAST_EXEC_NS
    feat = np.asarray(feat, np.float32)
    src0 = np.asarray(src0, np.int64); dst0 = np.asarray(dst0, np.int64)
    src1 = np.asarray(src1, np.int64); dst1 = np.asarray(dst1, np.int64)
    per_core, shared, node_of_slot, meta = _host_prep(
        feat, src0, dst0, src1, dst1,
        np.asarray(W_ih, np.float32), np.asarray(W_hh, np.float32),
        np.asarray(b_ih, np.float32), np.asarray(b_hh, np.float32),
    )
    nc = _build_nc(meta)
    in_maps = [{**shared, **pc} for pc in per_core]
    from concourse.bass_utils import run_bass_kernel_spmd
    if TRACE:
        import shutil
        shutil.rmtree("/tmp/trace_out2", ignore_errors=True)
    res = run_bass_kernel_spmd(
        nc, in_maps, list(range(NCORES)), trace=TRACE,
        tmpdir="/tmp/trace_out2" if TRACE else None,
    )
    LAST_EXEC_NS = res.exec_time_ns
    out = np.zeros((N_NODES, H), np.float32)
    for c in range(NCORES):
        oT = res.results[c]["outT"]          # [128, SLOTS]
        valid = node_of_slot[c] >= 0
        nodes = node_of_slot[c][valid]
        out[nodes] = oT.T[valid]
    return out


# revision 5
# speedup vs baseline: 1.0841x; 1.0008x over previous
"""Trainium2 Bass kernel for nn_LstmConv (GNN message passing + LSTMCell), v2.

Architecture (per core, dst-node sharding):
- Edges sorted into two continuous position streams (one per feature-table
  half), ordered by (tile, etype, slot); cells (tile, etype, half) padded to
  16-granular sizes common across cores (SPMD uniformity), ~4% pad.
- Streams gathered with 1024-index SWDGE dma_gather windows, 4 queues, deep
  buffer pools so the DMA rings never starve (descriptor-rate-bound at
  ~2.3ns/desc).
- One-hot matrices are pure 0/1, built by a single DVE is_equal pass per
  window (doff column per piece; a piece = chunk x cell intersection).
- Per (tile, etype) PSUM accumulates gb^T @ hot pieces; epilogue applies
  per-slot scales a_e[slot] (folding 1/deg and 1/n_active) and computes the
  LSTMCell exactly as the baseline.
"""

import sys, os

sys.path.insert(0, "/opt/trn_rl_repo")
sys.path.insert(0, os.path.dirname(os.path.abspath(__file__)))

import numpy as np
from ml_dtypes import bfloat16

N_NODES = 50000
N_EDGES = 800000
H = 128
MSG = 64
P = 128
NCORES = 8
TILES = 49
SLOTS = TILES * P          # 6272 per core
HALF = 25000
WIN = 1024                 # indices per gather window
CELL_GRAN = 8

LAST_EXEC_NS = None
TRACE = False


def _wrap16(idx):
    """[1024] -> [128, 64] wrap: idx j at [j%16, j//16], replicated x8."""
    n = len(idx)
    cols = (n + 15) // 16
    blk = np.zeros((16, cols), np.uint16)
    flat = np.zeros(cols * 16, np.uint16)
    flat[:n] = idx
    blk[:, :] = flat.reshape(cols, 16).T
    return np.tile(blk, (8, 1))


def _host_prep(feat, src0, dst0, src1, dst1, W_ih, W_hh, b_ih, b_hh):
    deg0 = np.bincount(dst0, minlength=N_NODES)
    deg1 = np.bincount(dst1, minlength=N_NODES)
    w = deg0 + deg1

    # snake-assign nodes (sorted by degree desc) into 392 tiles of <=128
    n_tiles_g = NCORES * TILES
    order = np.argsort(-w, kind="stable")
    tile_of_node = np.empty(N_NODES, np.int32)
    pos_in_tile = np.empty(N_NODES, np.int32)
    tcnt = np.zeros(n_tiles_g, np.int32)
    idx = 0
    fwd = True
    while idx < N_NODES:
        rng = range(n_tiles_g) if fwd else range(n_tiles_g - 1, -1, -1)
        for t in rng:
            if idx >= N_NODES:
                break
            if tcnt[t] < P:
                tile_of_node[order[idx]] = t
                pos_in_tile[order[idx]] = tcnt[t]
                tcnt[t] += 1
                idx += 1
        fwd = not fwd

    # balance tiles over cores by weight
    tile_w = np.zeros(n_tiles_g, np.int64)
    np.add.at(tile_w, tile_of_node, w)
    torder = np.argsort(-tile_w, kind="stable")
    core_of_tile = np.empty(n_tiles_g, np.int32)
    tl_of_tile = np.empty(n_tiles_g, np.int32)
    k = 0
    fwd = True
    for rnd in range(TILES):
        cr = range(NCORES) if fwd else range(NCORES - 1, -1, -1)
        for c in cr:
            core_of_tile[torder[k]] = c
            tl_of_tile[torder[k]] = rnd
            k += 1
        fwd = not fwd

    core_of_node = core_of_tile[tile_of_node]
    slot_of_node = tl_of_tile[tile_of_node] * P + pos_in_tile

    # rebalance nodes within each tile round across the 8 cores so the
    # per-(tile, etype, half) cell counts (whose max-over-cores sets the
    # padded cell size) are as even as possible
    d4 = np.zeros((N_NODES, 4), np.int64)       # (etype, half) edge counts
    for j, (s_, t_) in enumerate([(src0, dst0), (src1, dst1)]):
        for hh in range(2):
            mm = (s_ >= HALF) == (hh == 1)
            np.add.at(d4[:, j * 2 + hh], t_[mm], 1)
    rnd_of_node = tl_of_tile[tile_of_node]
    for r in range(TILES):
        nodes = np.where(rnd_of_node == r)[0]
        nodes = nodes[np.argsort(-w[nodes], kind="stable")]
        cnt8 = np.zeros((NCORES, 4), np.int64)
        fill = np.zeros(NCORES, np.int64)
        wsum = np.zeros(NCORES, np.int64)
        for n in nodes:
            best, bcost = -1, None
            mx = cnt8.max(axis=0)
            for c in range(NCORES):
                if fill[c] >= P:
                    continue
                inc = np.maximum(cnt8[c] + d4[n] - mx, 0).sum()
                cost = (inc, wsum[c])
                if bcost is None or cost < bcost:
                    best, bcost = c, cost
            cnt8[best] += d4[n]
            wsum[best] += w[n]
            core_of_node[n] = best
            slot_of_node[n] = r * P + fill[best]
            fill[best] += 1

    node_of_slot = -np.ones((NCORES, SLOTS), np.int64)
    node_of_slot[core_of_node, slot_of_node] = np.arange(N_NODES)

    # per-node scales: a_e = 1/max(deg_e,1) / max(has0+has1,1)
    has0 = (deg0 > 0).astype(np.float32)
    has1 = (deg1 > 0).astype(np.float32)
    invc = 1.0 / np.maximum(has0 + has1, 1.0)
    a0 = (invc / np.maximum(deg0, 1.0)).astype(np.float32)
    a1 = (invc / np.maximum(deg1, 1.0)).astype(np.float32)

    # edge table
    src_all = np.concatenate([src0, src1])
    dst_all = np.concatenate([dst0, dst1])
    et_all = np.concatenate([np.zeros(len(src0), np.int64),
                             np.ones(len(src1), np.int64)])
    c_all = core_of_node[dst_all]
    s_all = slot_of_node[dst_all]
    t_all = s_all // P
    sp_all = s_all % P
    h_all = (src_all >= HALF).astype(np.int64)
    srel_all = src_all - HALF * h_all

    # cell counts [core, tile, etype, half]
    cell_id = ((t_all * 2 + et_all) * 2 + h_all)
    cnt = np.zeros((NCORES, TILES * 4), np.int64)
    np.add.at(cnt, (c_all, cell_id), 1)
    CS = cnt.max(axis=0)                     # [TILES*4]
    CS = np.maximum((CS + CELL_GRAN - 1) // CELL_GRAN * CELL_GRAN, CELL_GRAN)
    CS = CS.reshape(TILES, 2, 2)             # [tile, etype, half]

    # stream layout per half: cells in (tile, etype) order
    stream_len = [0, 0]
    cell_off = np.zeros((TILES, 2, 2), np.int64)
    for t in range(TILES):
        for e in range(2):
            for h in range(2):
                cell_off[t, e, h] = stream_len[h]
                stream_len[h] += CS[t, e, h]
    # pad stream lengths to WIN multiple (extend last cell)
    for h in range(2):
        L = (stream_len[h] + WIN - 1) // WIN * WIN
        CS[TILES - 1, 1, h] += L - stream_len[h]
        stream_len[h] = L
    LA, LB = stream_len

    # piece table (identical across cores): walk each half-stream
    # piece = (half, window, chunk_in_win, tile, etype, pos_lo, pos_hi)
    pieces = []            # in per-half stream order
    for h in range(2):
        bounds = []        # (stream_pos_end, tile, etype)
        for t in range(TILES):
            for e in range(2):
                bounds.append((cell_off[t, e, h] + CS[t, e, h], t, e))
        L = stream_len[h]
        ci = 0
        pos = 0
        while pos < L:
            chunk_end = pos - pos % P + P
            cell_end, t, e = bounds[ci]
            hi = min(chunk_end, cell_end)
            pieces.append((h, pos // WIN, (pos % WIN) // P, t, e, pos, hi))
            if hi == cell_end and ci + 1 < len(bounds):
                ci += 1
            pos = hi
    NPIECES = len(pieces)

    # program-order interleave of windows: merge by tile progress so the two
    # streams stay within ~1 window of each other in tile space (bounds the
    # number of live PSUM tiles)
    nwin = [LA // WIN, LB // WIN]
    first_tile = {}
    for (h, wi, ck, t, e, lo, hi) in pieces:
        if (h, wi) not in first_tile:
            first_tile[(h, wi)] = t
    win_order = sorted(
        [(h, w) for h in range(2) for w in range(nwin[h])],
        key=lambda hw: (first_tile.get(hw, TILES), hw[1], hw[0]),
    )

    # pieces grouped by (half, window)
    pieces_by_win = {}
    for pi, pc in enumerate(pieces):
        pieces_by_win.setdefault((pc[0], pc[1]), []).append(pi)

    # start/stop flags per (tile, etype) in program order
    order_of_win = {hw: i for i, hw in enumerate(win_order)}
    first_piece = {}
    last_piece = {}
    for pi, (h, wi, ck, t, e, lo, hi) in enumerate(pieces):
        key = (t, e)
        rank = (order_of_win[(h, wi)], pi)
        if key not in first_piece or rank < first_piece[key][0]:
            first_piece[key] = (rank, pi)
        if key not in last_piece or rank > last_piece[key][0]:
            last_piece[key] = (rank, pi)
    start_of = {v[1] for v in first_piece.values()}
    stop_of = {v[1]: k for k, v in last_piece.items()}

    # per-core stream fill
    per_core = []
    for c in range(NCORES):
        m = c_all == c
        key = (h_all[m], t_all[m], et_all[m], sp_all[m])
        o = np.lexsort((key[3], key[2], key[1], key[0]))
        hs, ts, es, sps, srs = (h_all[m][o], t_all[m][o], et_all[m][o],
                                sp_all[m][o], srel_all[m][o])
        sidx = [np.zeros(LA, np.uint16), np.zeros(LB, np.uint16)]
        sdoff = [np.full(LA, 255.0, np.float32), np.full(LB, 255.0, np.float32)]
        # fill cells (edges sorted by (h, t, e, slot))
        cellk = (hs * TILES + ts) * 2 + es
        ccnt = np.bincount(cellk, minlength=TILES * 4)
        cbnd = np.concatenate([[0], np.cumsum(ccnt)])
        for h in range(2):
            for t in range(TILES):
                for e in range(2):
                    kk = (h * TILES + t) * 2 + e
                    lo2, hi2 = cbnd[kk], cbnd[kk + 1]
                    n = hi2 - lo2
                    off = cell_off[t, e, h]
                    assert n <= CS[t, e, h], (c, t, e, h, n, CS[t, e, h])
                    sidx[h][off:off + n] = srs[lo2:hi2]
                    sdoff[h][off:off + n] = sps[lo2:hi2]
        # gidx: wrap16 per window, concatenated in window ISSUE order so the
        # k-th slice of the gidx DMA covers the k-th issued gathers
        gw = [_wrap16(sidx[h][w * WIN:(w + 1) * WIN]) for (h, w) in win_order]
        gidx16 = np.concatenate(gw, axis=1).view(np.int16).copy()
        # doffP [128, NPIECES]
        doffP = np.full((P, NPIECES), 255.0, np.float32)
        for pi, (h, wi, ck, t, e, lo, hi) in enumerate(pieces):
            v = sdoff[h][lo:hi]
            doffP[lo % P:(lo % P) + (hi - lo), pi] = v
        # local node features, transposed, bf16
        sl = node_of_slot[c]
        floc = np.zeros((SLOTS, H), np.float32)
        floc[sl >= 0] = feat[sl[sl >= 0]]
        # per-slot scale tables, replicated across partitions
        a0s = np.zeros(SLOTS, np.float32)
        a1s = np.zeros(SLOTS, np.float32)
        a0s[sl >= 0] = a0[sl[sl >= 0]]
        a1s[sl >= 0] = a1[sl[sl >= 0]]
        a0rep = np.tile(a0s[None, :], (P, 1)).astype(bfloat16)
        a1rep = np.tile(a1s[None, :], (P, 1)).astype(bfloat16)
        per_core.append(dict(
            gidx16=gidx16, doffP=doffP.astype(bfloat16),
            a0rep=a0rep, a1rep=a1rep,
            featloc=floc.T.astype(bfloat16).copy()))

    featA = feat[:HALF].astype(bfloat16)
    featB = np.zeros((HALF, H), np.float32)
    featB[: N_NODES - HALF] = feat[HALF:]
    featB = featB.astype(bfloat16)

    wih = W_ih.T.astype(bfloat16).copy()              # [128, 256]
    whh = W_hh.T.astype(bfloat16).copy()              # [64, 256]
    bt = (b_ih + b_hh).astype(np.float32)
    biasT = np.stack([bt[:128], bt[128:]], axis=1).copy()  # [128, 2]
    iota = np.tile(np.arange(P, dtype=np.float32)[None, :], (P, 1)).astype(bfloat16)

    shared = dict(featA=featA, featB=featB, wih=wih, whh=whh, biasT=biasT,
                  iota=iota)
    meta = dict(pieces=pieces, pieces_by_win=pieces_by_win,
                win_order=win_order, nwin=nwin, NPIECES=NPIECES,
                start_of=start_of, stop_of=stop_of,
                GW=(nwin[0] + nwin[1]) * (WIN // 16))
    return per_core, shared, node_of_slot, meta


_WS = [0]


def _split_multi_waits(nc, mybir, max_waits=1):
    for fn in nc.m.functions:
        for bb in fn.blocks:
            new = []
            for ins in bb.instructions:
                si = ins.sync_info
                if si is not None and len(si.on_wait) > max_waits:
                    waits = list(si.on_wait)
                    for w in waits[:-max_waits]:
                        _WS[0] += 1
                        nop = mybir.InstNoOp(
                            name=f"I-waitsplit-{_WS[0]}", ins=[], outs=[]
                        )
                        nop.engine = ins.engine
                        nop.sync_info = mybir.SyncInfo(on_wait=[w], on_update=[])
                        new.append(nop)
                    si.on_wait = waits[-max_waits:]
                new.append(ins)
            bb.instructions[:] = new


def _build_nc(meta):
    from concourse import bass, mybir, tile, library_config

    f32, bf16, i16 = mybir.dt.float32, mybir.dt.bfloat16, mybir.dt.int16
    pieces = meta["pieces"]
    pieces_by_win = meta["pieces_by_win"]
    win_order = meta["win_order"]
    nwin = meta["nwin"]
    NPIECES = meta["NPIECES"]
    start_of = meta["start_of"]
    stop_of = meta["stop_of"]
    GW = meta["GW"]

    nc = bass.Bass(num_swdge_queues=4)
    featA_d = nc.declare_dram_parameter("featA", [HALF, H], bf16, isOutput=False)
    featB_d = nc.declare_dram_parameter("featB", [HALF, H], bf16, isOutput=False)
    gidx_d = nc.declare_dram_parameter("gidx16", [P, GW], i16, isOutput=False)
    doff_d = nc.declare_dram_parameter("doffP", [P, NPIECES], bf16, isOutput=False)
    a0_d = nc.declare_dram_parameter("a0rep", [P, SLOTS], bf16, isOutput=False)
    a1_d = nc.declare_dram_parameter("a1rep", [P, SLOTS], bf16, isOutput=False)
    wih_d = nc.declare_dram_parameter("wih", [P, 256], bf16, isOutput=False)
    whh_d = nc.declare_dram_parameter("whh", [64, 256], bf16, isOutput=False)
    bias_d = nc.declare_dram_parameter("biasT", [P, 2], f32, isOutput=False)
    iota_d = nc.declare_dram_parameter("iota", [P, P], bf16, isOutput=False)
    floc_d = nc.declare_dram_parameter("featloc", [P, SLOTS], bf16, isOutput=False)
    outT_d = nc.declare_dram_parameter("outT", [P, SLOTS], f32, isOutput=True)

    # window index base (into gidx cols) per (half, wi): issue order
    gidx_col = {}
    col = 0
    for hw in win_order:
        gidx_col[hw] = col
        col += WIN // 16

    with tile.TileContext(nc) as tc:
        with (
            tc.tile_pool(name="const", bufs=1) as cp,
            tc.tile_pool(name="gba", bufs=10) as gba,
            tc.tile_pool(name="gbb", bufs=10) as gbb,
            tc.tile_pool(name="hot", bufs=6) as hp,
            tc.tile_pool(name="ep", bufs=2) as ep,
            tc.tile_pool(name="psm", bufs=2, space="PSUM") as psm,
            tc.tile_pool(name="psg", bufs=1, space="PSUM") as psgp,
        ):
            nc.gpsimd.load_library(library_config.mlp)
            nireg = nc.gpsimd.to_reg(WIN)
            # gidx in separate slice tiles so the first gathers start
            # almost immediately (each gather depends only on its slice)
            NSL = 8
            sl = ((GW + NSL - 1) // NSL + 63) // 64 * 64
            gidx_sl = []
            for s in range(0, GW, sl):
                e_ = min(s + sl, GW)
                g_t = cp.tile([P, e_ - s], i16, name=f"gidx{s}")
                gidx_sl.append((s, e_, g_t))
                nc.sync.dma_start(out=g_t[:], in_=gidx_d[:, s:e_])
                if s == 0:
                    doff = cp.tile([P, NPIECES], bf16)
                    nc.sync.dma_start(out=doff[:], in_=doff_d[:])
                    iota = cp.tile([P, P], bf16)
                    nc.sync.dma_start(out=iota[:], in_=iota_d[:])

            def gidx_ap(c0, c1):
                for s, e_, g_t in gidx_sl:
                    if c0 >= s and c1 <= e_:
                        return g_t[:, c0 - s:c1 - s]
                raise AssertionError((c0, c1))
            a0rep = cp.tile([P, SLOTS], bf16)
            nc.sync.dma_start(out=a0rep[:], in_=a0_d[:])
            a1rep = cp.tile([P, SLOTS], bf16)
            nc.sync.dma_start(out=a1rep[:], in_=a1_d[:])
            wih = cp.tile([P, 256], bf16)
            nc.sync.dma_start(out=wih[:], in_=wih_d[:])
            whh = cp.tile([64, 256], bf16)
            nc.sync.dma_start(out=whh[:], in_=whh_d[:])
            bias = cp.tile([P, 2], f32)
            nc.sync.dma_start(out=bias[:], in_=bias_d[:])
            featloc = cp.tile([P, SLOTS], bf16)
            nc.sync.dma_start(out=featloc[:], in_=floc_d[:])

            gb_tiles = {}     # (half, wi) -> tile
            hot_tiles = {}    # (half, wi) -> (tile, piece_lo)
            pm_tiles = {}     # (tile, etype) -> psum tile

            qctr = [0]

            def issue_gather(h, wi):
                pool = gba if h == 0 else gbb
                gb = pool.tile([P, WIN // P, P], bf16, tag="gb")
                gb_tiles[(h, wi)] = gb
                nc.gpsimd.dma_gather(
                    out_ap=gb[:],
                    in_ap=(featA_d if h == 0 else featB_d)[:],
                    idxs_ap=gidx_ap(gidx_col[(h, wi)],
                                    gidx_col[(h, wi)] + WIN // 16),
                    num_idxs=WIN,
                    num_idxs_reg=nireg,
                    elem_size=H,
                    queue_num=qctr[0] % 4,
                )
                qctr[0] += 1

            PREFETCH = 8
            for k in range(min(PREFETCH, len(win_order))):
                issue_gather(*win_order[k])

            done_tiles = set()
            # count remaining stop flags per tile to trigger epilogue
            stops_needed = {}
            for pi, te in stop_of.items():
                stops_needed.setdefault(te[0], set()).add(te[1])

            def lstm_tile(tl, rstb):
                pg = []
                for half in range(2):
                    g_ps = psgp.tile([P, P], f32, tag=f"pg{half}")
                    nc.tensor.matmul(
                        out=g_ps[:], lhsT=wih[:, half * P:(half + 1) * P],
                        rhs=featloc[:, tl * P:(tl + 1) * P],
                        start=True, stop=False,
                    )
                    nc.tensor.matmul(
                        out=g_ps[:], lhsT=whh[:, half * P:(half + 1) * P],
                        rhs=rstb[0:64, :], start=False, stop=True,
                    )
                    pg.append(g_ps)
                sif = ep.tile([P, P], f32, tag="sif")
                nc.scalar.activation(
                    out=sif[:], in_=pg[0][:],
                    func=mybir.ActivationFunctionType.Sigmoid, bias=bias[:, 0:1],
                )
                sog = ep.tile([P, P], f32, tag="sog")
                nc.scalar.activation(
                    out=sog[0:64, :], in_=pg[1][0:64, :],
                    func=mybir.ActivationFunctionType.Tanh, bias=bias[0:64, 1:2],
                )
                nc.scalar.activation(
                    out=sog[64:128, :], in_=pg[1][64:128, :],
                    func=mybir.ActivationFunctionType.Sigmoid, bias=bias[64:128, 1:2],
                )
                outsb = ep.tile([P, P], f32, tag="outsb")
                t2 = ep.tile([64, P], f32, tag="t2")
                tt = ep.tile([P, P], f32, tag="tt")
                nc.vector.tensor_tensor(
                    out=t2[:], in0=sif[0:64, :], in1=sog[0:64, :],
                    op=mybir.AluOpType.mult,
                )
                nc.scalar.activation(
                    out=tt[64:128, :], in_=t2[:],
                    func=mybir.ActivationFunctionType.Copy,
                )
                nc.vector.tensor_tensor(
                    out=outsb[64:128, :], in0=sif[64:128, :], in1=rstb[64:128, :],
                    op=mybir.AluOpType.mult,
                )
                nc.vector.tensor_tensor(
                    out=outsb[64:128, :], in0=outsb[64:128, :], in1=tt[64:128, :],
                    op=mybir.AluOpType.add,
                )
                nc.scalar.activation(
                    out=tt[64:128, :], in_=outsb[64:128, :],
                    func=mybir.ActivationFunctionType.Tanh,
                )
                nc.vector.tensor_tensor(
                    out=tt[64:128, :], in0=sog[64:128, :], in1=tt[64:128, :],
                    op=mybir.AluOpType.mult,
                )
                nc.scalar.activation(
                    out=outsb[0:64, :], in_=tt[64:128, :],
                    func=mybir.ActivationFunctionType.Copy,
                )
                nc.sync.dma_start(
                    out=outT_d[:, tl * P:(tl + 1) * P], in_=outsb[:]
                )

            for k, (h, wi) in enumerate(win_order):
                if k + PREFETCH < len(win_order):
                    issue_gather(*win_order[k + PREFETCH])
                plist = pieces_by_win.get((h, wi), [])
                if not plist:
                    continue
                p_lo = plist[0]
                npz = len(plist)
                # one is_equal builds all hots of this window
                hot = hp.tile([P, npz, P], bf16, tag="hot")
                nc.vector.tensor_tensor(
                    out=hot[:],
                    in0=doff[:, p_lo:p_lo + npz].to_broadcast([P, npz, P]),
                    in1=iota[:, None, :].to_broadcast([P, npz, P]),
                    op=mybir.AluOpType.is_equal,
                )
                gb = gb_tiles.pop((h, wi))
                for j, pi in enumerate(plist):
                    ph, pwi, ck, t, e, lo, hi = pieces[pi]
                    key = (t, e)
                    if pi in start_of:
                        pm_tiles[key] = psm.tile(
                            [P, P], f32, tag=f"pm{e}", name=f"pm{e}_{t}")
                    nc.tensor.matmul(
                        out=pm_tiles[key][:],
                        lhsT=gb[:, ck, :],
                        rhs=hot[:, j, :],
                        start=(pi in start_of),
                        stop=(pi in stop_of),
                    )
                    if pi in stop_of:
                        te = stop_of[pi]
                        tl = te[0]
                        stops_needed[tl].discard(te[1])
                        if not stops_needed[tl]:
                            # epilogue: rstb = pm0*a0 + pm1*a1 (bf16)
                            pm0 = pm_tiles.pop((tl, 0))
                            pm1 = pm_tiles.pop((tl, 1))
                            tta = ep.tile([P, P], f32, tag="tta")
                            nc.vector.tensor_tensor(
                                out=tta[:], in0=pm0[:],
                                in1=a0rep[:, tl * P:(tl + 1) * P],
                                op=mybir.AluOpType.mult,
                            )
                            ttb = ep.tile([P, P], f32, tag="ttb")
                            nc.vector.tensor_tensor(
                                out=ttb[:], in0=pm1[:],
                                in1=a1rep[:, tl * P:(tl + 1) * P],
                                op=mybir.AluOpType.mult,
                            )
                            rstb = ep.tile([P, P], bf16, tag="rstb")
                            nc.vector.tensor_tensor(
                                out=rstb[:], in0=tta[:], in1=ttb[:],
                                op=mybir.AluOpType.add,
                            )
                            lstm_tile(tl, rstb)

    from concourse import mybir as _mb
    _mb.codegen_inst_isa_subclasses(nc)
    _split_multi_waits(nc, mybir)
    return nc


def kernel(feat, src0, dst0, src1, dst1, W_ih, W_hh, b_ih, b_hh):
    global LAST_EXEC_NS
    feat = np.asarray(feat, np.float32)
    src0 = np.asarray(src0, np.int64); dst0 = np.asarray(dst0, np.int64)
    src1 = np.asarray(src1, np.int64); dst1 = np.asarray(dst1, np.int64)
    per_core, shared, node_of_slot, meta = _host_prep(
        feat, src0, dst0, src1, dst1,
        np.asarray(W_ih, np.float32), np.asarray(W_hh, np.float32),
        np.asarray(b_ih, np.float32), np.asarray(b_hh, np.float32),
    )
    nc = _build_nc(meta)
    in_maps = [{**shared, **pc} for pc in per_core]
    from concourse.bass_utils import run_bass_kernel_spmd
    if TRACE:
        import shutil
        shutil.rmtree("/tmp/trace_out2", ignore_errors=True)
    res = run_bass_kernel_spmd(
        nc, in_maps, list(range(NCORES)), trace=TRACE,
        tmpdir="/tmp/trace_out2" if TRACE else None,
    )
    LAST_EXEC_NS = res.exec_time_ns
    out = np.zeros((N_NODES, H), np.float32)
    for c in range(NCORES):
        oT = res.results[c]["outT"]          # [128, SLOTS]
        valid = node_of_slot[c] >= 0
        nodes = node_of_slot[c][valid]
        out[nodes] = oT.T[valid]
    return out


# revision 6
# speedup vs baseline: 1.0905x; 1.0059x over previous
"""Trainium2 Bass kernel for nn_LstmConv (GNN message passing + LSTMCell), v2.

Architecture (per core, dst-node sharding):
- Edges sorted into two continuous position streams (one per feature-table
  half), ordered by (tile, etype, slot); cells (tile, etype, half) padded to
  16-granular sizes common across cores (SPMD uniformity), ~4% pad.
- Streams gathered with 1024-index SWDGE dma_gather windows, 4 queues, deep
  buffer pools so the DMA rings never starve (descriptor-rate-bound at
  ~2.3ns/desc).
- One-hot matrices are pure 0/1, built by a single DVE is_equal pass per
  window (doff column per piece; a piece = chunk x cell intersection).
- Per (tile, etype) PSUM accumulates gb^T @ hot pieces; epilogue applies
  per-slot scales a_e[slot] (folding 1/deg and 1/n_active) and computes the
  LSTMCell exactly as the baseline.
"""

import sys, os

sys.path.insert(0, "/opt/trn_rl_repo")
sys.path.insert(0, os.path.dirname(os.path.abspath(__file__)))

import numpy as np
from ml_dtypes import bfloat16

N_NODES = 50000
N_EDGES = 800000
H = 128
MSG = 64
P = 128
NCORES = 8
TILES = 49
SLOTS = TILES * P          # 6272 per core
HALF = 25000
WIN = 1024                 # indices per gather window
CELL_GRAN = 8

LAST_EXEC_NS = None
TRACE = False


def _wrap16(idx, width=WIN // 16):
    """-> [128, width] wrap: idx j at [j%16, j//16], replicated x8."""
    n = len(idx)
    blk = np.zeros((16, width), np.uint16)
    flat = np.zeros(width * 16, np.uint16)
    flat[:n] = idx
    blk[:, :] = flat.reshape(width, 16).T
    return np.tile(blk, (8, 1))


def _host_prep(feat, src0, dst0, src1, dst1, W_ih, W_hh, b_ih, b_hh):
    deg0 = np.bincount(dst0, minlength=N_NODES)
    deg1 = np.bincount(dst1, minlength=N_NODES)
    w = deg0 + deg1

    # snake-assign nodes (sorted by degree desc) into 392 tiles of <=128
    n_tiles_g = NCORES * TILES
    order = np.argsort(-w, kind="stable")
    tile_of_node = np.empty(N_NODES, np.int32)
    pos_in_tile = np.empty(N_NODES, np.int32)
    tcnt = np.zeros(n_tiles_g, np.int32)
    idx = 0
    fwd = True
    while idx < N_NODES:
        rng = range(n_tiles_g) if fwd else range(n_tiles_g - 1, -1, -1)
        for t in rng:
            if idx >= N_NODES:
                break
            if tcnt[t] < P:
                tile_of_node[order[idx]] = t
                pos_in_tile[order[idx]] = tcnt[t]
                tcnt[t] += 1
                idx += 1
        fwd = not fwd

    # balance tiles over cores by weight
    tile_w = np.zeros(n_tiles_g, np.int64)
    np.add.at(tile_w, tile_of_node, w)
    torder = np.argsort(-tile_w, kind="stable")
    core_of_tile = np.empty(n_tiles_g, np.int32)
    tl_of_tile = np.empty(n_tiles_g, np.int32)
    k = 0
    fwd = True
    for rnd in range(TILES):
        cr = range(NCORES) if fwd else range(NCORES - 1, -1, -1)
        for c in cr:
            core_of_tile[torder[k]] = c
            tl_of_tile[torder[k]] = rnd
            k += 1
        fwd = not fwd

    core_of_node = core_of_tile[tile_of_node]
    slot_of_node = tl_of_tile[tile_of_node] * P + pos_in_tile

    # rebalance nodes within each tile round across the 8 cores so the
    # per-(tile, etype, half) cell counts (whose max-over-cores sets the
    # padded cell size) are as even as possible
    d4 = np.zeros((N_NODES, 4), np.int64)       # (etype, half) edge counts
    for j, (s_, t_) in enumerate([(src0, dst0), (src1, dst1)]):
        for hh in range(2):
            mm = (s_ >= HALF) == (hh == 1)
            np.add.at(d4[:, j * 2 + hh], t_[mm], 1)
    rnd_of_node = tl_of_tile[tile_of_node]
    for r in range(TILES):
        nodes = np.where(rnd_of_node == r)[0]
        nodes = nodes[np.argsort(-w[nodes], kind="stable")]
        cnt8 = np.zeros((NCORES, 4), np.int64)
        fill = np.zeros(NCORES, np.int64)
        wsum = np.zeros(NCORES, np.int64)
        for n in nodes:
            best, bcost = -1, None
            mx = cnt8.max(axis=0)
            for c in range(NCORES):
                if fill[c] >= P:
                    continue
                inc = np.maximum(cnt8[c] + d4[n] - mx, 0).sum()
                cost = (inc, wsum[c])
                if bcost is None or cost < bcost:
                    best, bcost = c, cost
            cnt8[best] += d4[n]
            wsum[best] += w[n]
            core_of_node[n] = best
            slot_of_node[n] = r * P + fill[best]
            fill[best] += 1

    node_of_slot = -np.ones((NCORES, SLOTS), np.int64)
    node_of_slot[core_of_node, slot_of_node] = np.arange(N_NODES)

    # per-node scales: a_e = 1/max(deg_e,1) / max(has0+has1,1)
    has0 = (deg0 > 0).astype(np.float32)
    has1 = (deg1 > 0).astype(np.float32)
    invc = 1.0 / np.maximum(has0 + has1, 1.0)
    a0 = (invc / np.maximum(deg0, 1.0)).astype(np.float32)
    a1 = (invc / np.maximum(deg1, 1.0)).astype(np.float32)

    # edge table
    src_all = np.concatenate([src0, src1])
    dst_all = np.concatenate([dst0, dst1])
    et_all = np.concatenate([np.zeros(len(src0), np.int64),
                             np.ones(len(src1), np.int64)])
    c_all = core_of_node[dst_all]
    s_all = slot_of_node[dst_all]
    t_all = s_all // P
    sp_all = s_all % P
    h_all = (src_all >= HALF).astype(np.int64)
    srel_all = src_all - HALF * h_all

    # cell counts [core, tile, etype, half]
    cell_id = ((t_all * 2 + et_all) * 2 + h_all)
    cnt = np.zeros((NCORES, TILES * 4), np.int64)
    np.add.at(cnt, (c_all, cell_id), 1)
    CS = cnt.max(axis=0)                     # [TILES*4]
    CS = np.maximum((CS + CELL_GRAN - 1) // CELL_GRAN * CELL_GRAN, CELL_GRAN)
    CS = CS.reshape(TILES, 2, 2)             # [tile, etype, half]

    # stream layout per half: cells in (tile, etype) order
    stream_len = [0, 0]
    cell_off = np.zeros((TILES, 2, 2), np.int64)
    for t in range(TILES):
        for e in range(2):
            for h in range(2):
                cell_off[t, e, h] = stream_len[h]
                stream_len[h] += CS[t, e, h]
    # final window of each stream is short: only round up to a chunk (128)
    LA = (stream_len[0] + P - 1) // P * P
    LB = (stream_len[1] + P - 1) // P * P
    stream_len = [LA, LB]
    # gather size of window wi of half h
    nwin = [(LA + WIN - 1) // WIN, (LB + WIN - 1) // WIN]
    win_nidx = {}
    for h in range(2):
        for wi in range(nwin[h]):
            win_nidx[(h, wi)] = min(WIN, stream_len[h] - wi * WIN)

    # piece table (identical across cores): walk each half-stream
    # piece = (half, window, chunk_in_win, tile, etype, pos_lo, pos_hi)
    pieces = []            # in per-half stream order
    for h in range(2):
        bounds = []        # (stream_pos_end, tile, etype)
        for t in range(TILES):
            for e in range(2):
                bounds.append((cell_off[t, e, h] + CS[t, e, h], t, e))
        L = stream_len[h]
        bounds[-1] = (L, bounds[-1][1], bounds[-1][2])
        ci = 0
        pos = 0
        while pos < L:
            chunk_end = pos - pos % P + P
            cell_end, t, e = bounds[ci]
            hi = min(chunk_end, cell_end)
            pieces.append((h, pos // WIN, (pos % WIN) // P, t, e, pos, hi))
            if hi == cell_end and ci + 1 < len(bounds):
                ci += 1
            pos = hi
    NPIECES = len(pieces)

    # program-order interleave of windows: merge by tile progress so the two
    # streams stay within ~1 window of each other in tile space (bounds the
    # number of live PSUM tiles)
    first_tile = {}
    for (h, wi, ck, t, e, lo, hi) in pieces:
        if (h, wi) not in first_tile:
            first_tile[(h, wi)] = t
    win_order = sorted(
        [(h, w) for h in range(2) for w in range(nwin[h])],
        key=lambda hw: (first_tile.get(hw, TILES), hw[1], hw[0]),
    )

    # pieces grouped by (half, window)
    pieces_by_win = {}
    for pi, pc in enumerate(pieces):
        pieces_by_win.setdefault((pc[0], pc[1]), []).append(pi)

    # start/stop flags per (tile, etype) in program order
    order_of_win = {hw: i for i, hw in enumerate(win_order)}
    first_piece = {}
    last_piece = {}
    for pi, (h, wi, ck, t, e, lo, hi) in enumerate(pieces):
        key = (t, e)
        rank = (order_of_win[(h, wi)], pi)
        if key not in first_piece or rank < first_piece[key][0]:
            first_piece[key] = (rank, pi)
        if key not in last_piece or rank > last_piece[key][0]:
            last_piece[key] = (rank, pi)
    start_of = {v[1] for v in first_piece.values()}
    stop_of = {v[1]: k for k, v in last_piece.items()}

    # per-core stream fill
    per_core = []
    for c in range(NCORES):
        m = c_all == c
        key = (h_all[m], t_all[m], et_all[m], sp_all[m])
        o = np.lexsort((key[3], key[2], key[1], key[0]))
        hs, ts, es, sps, srs = (h_all[m][o], t_all[m][o], et_all[m][o],
                                sp_all[m][o], srel_all[m][o])
        sidx = [np.zeros(LA, np.uint16), np.zeros(LB, np.uint16)]
        sdoff = [np.full(LA, 255.0, np.float32), np.full(LB, 255.0, np.float32)]
        # fill cells (edges sorted by (h, t, e, slot))
        cellk = (hs * TILES + ts) * 2 + es
        ccnt = np.bincount(cellk, minlength=TILES * 4)
        cbnd = np.concatenate([[0], np.cumsum(ccnt)])
        for h in range(2):
            for t in range(TILES):
                for e in range(2):
                    kk = (h * TILES + t) * 2 + e
                    lo2, hi2 = cbnd[kk], cbnd[kk + 1]
                    n = hi2 - lo2
                    off = cell_off[t, e, h]
                    assert n <= CS[t, e, h], (c, t, e, h, n, CS[t, e, h])
                    sidx[h][off:off + n] = srs[lo2:hi2]
                    sdoff[h][off:off + n] = sps[lo2:hi2]
        # gidx: wrap16 per window, concatenated in window ISSUE order so the
        # k-th slice of the gidx DMA covers the k-th issued gathers
        gw = [_wrap16(sidx[h][w * WIN:(w + 1) * WIN]) for (h, w) in win_order]
        gidx16 = np.concatenate(gw, axis=1).view(np.int16).copy()
        # doffP [128, NPIECES]
        doffP = np.full((P, NPIECES), 255.0, np.float32)
        for pi, (h, wi, ck, t, e, lo, hi) in enumerate(pieces):
            v = sdoff[h][lo:hi]
            doffP[lo % P:(lo % P) + (hi - lo), pi] = v
        # local node features, transposed, bf16
        sl = node_of_slot[c]
        floc = np.zeros((SLOTS, H), np.float32)
        floc[sl >= 0] = feat[sl[sl >= 0]]
        # per-slot scale tables, replicated across partitions
        a0s = np.zeros(SLOTS, np.float32)
        a1s = np.zeros(SLOTS, np.float32)
        a0s[sl >= 0] = a0[sl[sl >= 0]]
        a1s[sl >= 0] = a1[sl[sl >= 0]]
        a0rep = np.tile(a0s[None, :], (P, 1)).astype(bfloat16)
        a1rep = np.tile(a1s[None, :], (P, 1)).astype(bfloat16)
        per_core.append(dict(
            gidx16=gidx16, doffP=doffP.astype(bfloat16),
            a0rep=a0rep, a1rep=a1rep,
            featloc=floc.T.astype(bfloat16).copy()))

    featA = feat[:HALF].astype(bfloat16)
    featB = np.zeros((HALF, H), np.float32)
    featB[: N_NODES - HALF] = feat[HALF:]
    featB = featB.astype(bfloat16)

    wih = W_ih.T.astype(bfloat16).copy()              # [128, 256]
    whh = W_hh.T.astype(bfloat16).copy()              # [64, 256]
    bt = (b_ih + b_hh).astype(np.float32)
    biasT = np.stack([bt[:128], bt[128:]], axis=1).copy()  # [128, 2]
    iota = np.tile(np.arange(P, dtype=np.float32)[None, :], (P, 1)).astype(bfloat16)

    shared = dict(featA=featA, featB=featB, wih=wih, whh=whh, biasT=biasT,
                  iota=iota)
    meta = dict(pieces=pieces, pieces_by_win=pieces_by_win,
                win_order=win_order, nwin=nwin, NPIECES=NPIECES,
                start_of=start_of, stop_of=stop_of, win_nidx=win_nidx,
                GW=(nwin[0] + nwin[1]) * (WIN // 16))
    return per_core, shared, node_of_slot, meta


_WS = [0]


def _split_multi_waits(nc, mybir, max_waits=1):
    for fn in nc.m.functions:
        for bb in fn.blocks:
            new = []
            for ins in bb.instructions:
                si = ins.sync_info
                if si is not None and len(si.on_wait) > max_waits:
                    waits = list(si.on_wait)
                    for w in waits[:-max_waits]:
                        _WS[0] += 1
                        nop = mybir.InstNoOp(
                            name=f"I-waitsplit-{_WS[0]}", ins=[], outs=[]
                        )
                        nop.engine = ins.engine
                        nop.sync_info = mybir.SyncInfo(on_wait=[w], on_update=[])
                        new.append(nop)
                    si.on_wait = waits[-max_waits:]
                new.append(ins)
            bb.instructions[:] = new


def _build_nc(meta):
    from concourse import bass, mybir, tile, library_config

    f32, bf16, i16 = mybir.dt.float32, mybir.dt.bfloat16, mybir.dt.int16
    pieces = meta["pieces"]
    pieces_by_win = meta["pieces_by_win"]
    win_order = meta["win_order"]
    nwin = meta["nwin"]
    win_nidx = meta["win_nidx"]
    NPIECES = meta["NPIECES"]
    start_of = meta["start_of"]
    stop_of = meta["stop_of"]
    GW = meta["GW"]

    nc = bass.Bass(num_swdge_queues=4)
    featA_d = nc.declare_dram_parameter("featA", [HALF, H], bf16, isOutput=False)
    featB_d = nc.declare_dram_parameter("featB", [HALF, H], bf16, isOutput=False)
    gidx_d = nc.declare_dram_parameter("gidx16", [P, GW], i16, isOutput=False)
    doff_d = nc.declare_dram_parameter("doffP", [P, NPIECES], bf16, isOutput=False)
    a0_d = nc.declare_dram_parameter("a0rep", [P, SLOTS], bf16, isOutput=False)
    a1_d = nc.declare_dram_parameter("a1rep", [P, SLOTS], bf16, isOutput=False)
    wih_d = nc.declare_dram_parameter("wih", [P, 256], bf16, isOutput=False)
    whh_d = nc.declare_dram_parameter("whh", [64, 256], bf16, isOutput=False)
    bias_d = nc.declare_dram_parameter("biasT", [P, 2], f32, isOutput=False)
    iota_d = nc.declare_dram_parameter("iota", [P, P], bf16, isOutput=False)
    floc_d = nc.declare_dram_parameter("featloc", [P, SLOTS], bf16, isOutput=False)
    outT_d = nc.declare_dram_parameter("outT", [P, SLOTS], f32, isOutput=True)

    # window index base (into gidx cols) per (half, wi): issue order
    gidx_col = {}
    col = 0
    for hw in win_order:
        gidx_col[hw] = col
        col += WIN // 16

    with tile.TileContext(nc) as tc:
        with (
            tc.tile_pool(name="const", bufs=1) as cp,
            tc.tile_pool(name="gba", bufs=10) as gba,
            tc.tile_pool(name="gbb", bufs=10) as gbb,
            tc.tile_pool(name="hot", bufs=6) as hp,
            tc.tile_pool(name="ep", bufs=2) as ep,
            tc.tile_pool(name="psm", bufs=2, space="PSUM") as psm,
            tc.tile_pool(name="psg", bufs=1, space="PSUM") as psgp,
        ):
            nc.gpsimd.load_library(library_config.mlp)
            niregs = {n: nc.gpsimd.to_reg(n) for n in sorted(set(win_nidx.values()))}
            # gidx in separate slice tiles so the first gathers start
            # almost immediately (each gather depends only on its slice)
            NSL = 8
            sl = ((GW + NSL - 1) // NSL + 63) // 64 * 64
            gidx_sl = []
            for s in range(0, GW, sl):
                e_ = min(s + sl, GW)
                g_t = cp.tile([P, e_ - s], i16, name=f"gidx{s}")
                gidx_sl.append((s, e_, g_t))
                nc.sync.dma_start(out=g_t[:], in_=gidx_d[:, s:e_])
                if s == 0:
                    doff = cp.tile([P, NPIECES], bf16)
                    nc.sync.dma_start(out=doff[:], in_=doff_d[:])
                    iota = cp.tile([P, P], bf16)
                    nc.sync.dma_start(out=iota[:], in_=iota_d[:])

            def gidx_ap(c0, c1):
                for s, e_, g_t in gidx_sl:
                    if c0 >= s and c1 <= e_:
                        return g_t[:, c0 - s:c1 - s]
                raise AssertionError((c0, c1))
            a0rep = cp.tile([P, SLOTS], bf16)
            nc.sync.dma_start(out=a0rep[:], in_=a0_d[:])
            a1rep = cp.tile([P, SLOTS], bf16)
            nc.sync.dma_start(out=a1rep[:], in_=a1_d[:])
            wih = cp.tile([P, 256], bf16)
            nc.sync.dma_start(out=wih[:], in_=wih_d[:])
            whh = cp.tile([64, 256], bf16)
            nc.sync.dma_start(out=whh[:], in_=whh_d[:])
            bias = cp.tile([P, 2], f32)
            nc.sync.dma_start(out=bias[:], in_=bias_d[:])
            featloc = cp.tile([P, SLOTS], bf16)
            nc.sync.dma_start(out=featloc[:], in_=floc_d[:])

            gb_tiles = {}     # (half, wi) -> tile
            hot_tiles = {}    # (half, wi) -> (tile, piece_lo)
            pm_tiles = {}     # (tile, etype) -> psum tile

            qctr = [0]

            def issue_gather(h, wi):
                pool = gba if h == 0 else gbb
                gb = pool.tile([P, WIN // P, P], bf16, tag="gb")
                gb_tiles[(h, wi)] = gb
                nn = win_nidx[(h, wi)]
                nc.gpsimd.dma_gather(
                    out_ap=gb[:, 0:nn // P, :],
                    in_ap=(featA_d if h == 0 else featB_d)[:],
                    idxs_ap=gidx_ap(gidx_col[(h, wi)],
                                    gidx_col[(h, wi)] + nn // 16),
                    num_idxs=nn,
                    num_idxs_reg=niregs[nn],
                    elem_size=H,
                    queue_num=qctr[0] % 4,
                )
                qctr[0] += 1

            PREFETCH = 8
            for k in range(min(PREFETCH, len(win_order))):
                issue_gather(*win_order[k])

            done_tiles = set()
            # count remaining stop flags per tile to trigger epilogue
            stops_needed = {}
            for pi, te in stop_of.items():
                stops_needed.setdefault(te[0], set()).add(te[1])

            def lstm_tile(tl, rstb):
                pg = []
                for half in range(2):
                    g_ps = psgp.tile([P, P], f32, tag=f"pg{half}")
                    nc.tensor.matmul(
                        out=g_ps[:], lhsT=wih[:, half * P:(half + 1) * P],
                        rhs=featloc[:, tl * P:(tl + 1) * P],
                        start=True, stop=False,
                    )
                    nc.tensor.matmul(
                        out=g_ps[:], lhsT=whh[:, half * P:(half + 1) * P],
                        rhs=rstb[0:64, :], start=False, stop=True,
                    )
                    pg.append(g_ps)
                sif = ep.tile([P, P], f32, tag="sif")
                nc.scalar.activation(
                    out=sif[:], in_=pg[0][:],
                    func=mybir.ActivationFunctionType.Sigmoid, bias=bias[:, 0:1],
                )
                sog = ep.tile([P, P], f32, tag="sog")
                nc.scalar.activation(
                    out=sog[0:64, :], in_=pg[1][0:64, :],
                    func=mybir.ActivationFunctionType.Tanh, bias=bias[0:64, 1:2],
                )
                nc.scalar.activation(
                    out=sog[64:128, :], in_=pg[1][64:128, :],
                    func=mybir.ActivationFunctionType.Sigmoid, bias=bias[64:128, 1:2],
                )
                outsb = ep.tile([P, P], f32, tag="outsb")
                t2 = ep.tile([64, P], f32, tag="t2")
                tt = ep.tile([P, P], f32, tag="tt")
                nc.vector.tensor_tensor(
                    out=t2[:], in0=sif[0:64, :], in1=sog[0:64, :],
                    op=mybir.AluOpType.mult,
                )
                nc.scalar.activation(
                    out=tt[64:128, :], in_=t2[:],
                    func=mybir.ActivationFunctionType.Copy,
                )
                nc.vector.tensor_tensor(
                    out=outsb[64:128, :], in0=sif[64:128, :], in1=rstb[64:128, :],
                    op=mybir.AluOpType.mult,
                )
                nc.vector.tensor_tensor(
                    out=outsb[64:128, :], in0=outsb[64:128, :], in1=tt[64:128, :],
                    op=mybir.AluOpType.add,
                )
                nc.scalar.activation(
                    out=tt[64:128, :], in_=outsb[64:128, :],
                    func=mybir.ActivationFunctionType.Tanh,
                )
                nc.vector.tensor_tensor(
                    out=tt[64:128, :], in0=sog[64:128, :], in1=tt[64:128, :],
                    op=mybir.AluOpType.mult,
                )
                nc.scalar.activation(
                    out=outsb[0:64, :], in_=tt[64:128, :],
                    func=mybir.ActivationFunctionType.Copy,
                )
                nc.sync.dma_start(
                    out=outT_d[:, tl * P:(tl + 1) * P], in_=outsb[:]
                )

            for k, (h, wi) in enumerate(win_order):
                if k + PREFETCH < len(win_order):
                    issue_gather(*win_order[k + PREFETCH])
                plist = pieces_by_win.get((h, wi), [])
                if not plist:
                    continue
                p_lo = plist[0]
                npz = len(plist)
                # one is_equal builds all hots of this window
                hot = hp.tile([P, npz, P], bf16, tag="hot")
                nc.vector.tensor_tensor(
                    out=hot[:],
                    in0=doff[:, p_lo:p_lo + npz].to_broadcast([P, npz, P]),
                    in1=iota[:, None, :].to_broadcast([P, npz, P]),
                    op=mybir.AluOpType.is_equal,
                )
                gb = gb_tiles.pop((h, wi))
                for j, pi in enumerate(plist):
                    ph, pwi, ck, t, e, lo, hi = pieces[pi]
                    key = (t, e)
                    if pi in start_of:
                        pm_tiles[key] = psm.tile(
                            [P, P], f32, tag=f"pm{e}", name=f"pm{e}_{t}")
                    nc.tensor.matmul(
                        out=pm_tiles[key][:],
                        lhsT=gb[:, ck, :],
                        rhs=hot[:, j, :],
                        start=(pi in start_of),
                        stop=(pi in stop_of),
                    )
                    if pi in stop_of:
                        te = stop_of[pi]
                        tl = te[0]
                        stops_needed[tl].discard(te[1])
                        if not stops_needed[tl]:
                            # epilogue: rstb = pm0*a0 + pm1*a1 (bf16)
                            pm0 = pm_tiles.pop((tl, 0))
                            pm1 = pm_tiles.pop((tl, 1))
                            tta = ep.tile([P, P], f32, tag="tta")
                            nc.vector.tensor_tensor(
                                out=tta[:], in0=pm0[:],
                                in1=a0rep[:, tl * P:(tl + 1) * P],
                                op=mybir.AluOpType.mult,
                            )
                            ttb = ep.tile([P, P], f32, tag="ttb")
                            nc.vector.tensor_tensor(
                                out=ttb[:], in0=pm1[:],
                                in1=a1rep[:, tl * P:(tl + 1) * P],
                                op=mybir.AluOpType.mult,
                            )
                            rstb = ep.tile([P, P], bf16, tag="rstb")
                            nc.vector.tensor_tensor(
                                out=rstb[:], in0=tta[:], in1=ttb[:],
                                op=mybir.AluOpType.add,
                            )
                            lstm_tile(tl, rstb)

    from concourse import mybir as _mb
    _mb.codegen_inst_isa_subclasses(nc)
    _split_multi_waits(nc, mybir)
    return nc


def kernel(feat, src0, dst0, src1, dst1, W_ih, W_hh, b_ih, b_hh):
    global LAST_EXEC_NS
    feat = np.asarray(feat, np.float32)
    src0 = np.asarray(src0, np.int64); dst0 = np.asarray(dst0, np.int64)
    src1 = np.asarray(src1, np.int64); dst1 = np.asarray(dst1, np.int64)
    per_core, shared, node_of_slot, meta = _host_prep(
        feat, src0, dst0, src1, dst1,
        np.asarray(W_ih, np.float32), np.asarray(W_hh, np.float32),
        np.asarray(b_ih, np.float32), np.asarray(b_hh, np.float32),
    )
    nc = _build_nc(meta)
    in_maps = [{**shared, **pc} for pc in per_core]
    from concourse.bass_utils import run_bass_kernel_spmd
    if TRACE:
        import shutil
        shutil.rmtree("/tmp/trace_out2", ignore_errors=True)
    res = run_bass_kernel_spmd(
        nc, in_maps, list(range(NCORES)), trace=TRACE,
        tmpdir="/tmp/trace_out2" if TRACE else None,
    )
    LAST_EXEC_NS = res.exec_time_ns
    out = np.zeros((N_NODES, H), np.float32)
    for c in range(NCORES):
        oT = res.results[c]["outT"]          # [128, SLOTS]
        valid = node_of_slot[c] >= 0
        nodes = node_of_slot[c][valid]
        out[nodes] = oT.T[valid]
    return out


# revision 7
# speedup vs baseline: 1.1075x; 1.0156x over previous
"""Trainium2 Bass kernel for nn_LstmConv (GNN message passing + LSTMCell), v2.

Architecture (per core, dst-node sharding):
- Edges sorted into two continuous position streams (one per feature-table
  half), ordered by (tile, etype, slot); cells (tile, etype, half) padded to
  16-granular sizes common across cores (SPMD uniformity), ~4% pad.
- Streams gathered with 1024-index SWDGE dma_gather windows, 4 queues, deep
  buffer pools so the DMA rings never starve (descriptor-rate-bound at
  ~2.3ns/desc).
- One-hot matrices are pure 0/1, built by a single DVE is_equal pass per
  window (doff column per piece; a piece = chunk x cell intersection).
- Per (tile, etype) PSUM accumulates gb^T @ hot pieces; epilogue applies
  per-slot scales a_e[slot] (folding 1/deg and 1/n_active) and computes the
  LSTMCell exactly as the baseline.
"""

import sys, os

sys.path.insert(0, "/opt/trn_rl_repo")
sys.path.insert(0, os.path.dirname(os.path.abspath(__file__)))

import numpy as np
from ml_dtypes import bfloat16

N_NODES = 50000
N_EDGES = 800000
H = 128
MSG = 64
P = 128
NCORES = 8
TILES = 49
SLOTS = TILES * P          # 6272 per core
HALF = 25000
WIN = 1024                 # indices per gather window
CELL_GRAN = 8

LAST_EXEC_NS = None
TRACE = False


def _wrap16(idx, width=WIN // 16):
    """-> [128, width] wrap: idx j at [j%16, j//16], replicated x8."""
    n = len(idx)
    blk = np.zeros((16, width), np.uint16)
    flat = np.zeros(width * 16, np.uint16)
    flat[:n] = idx
    blk[:, :] = flat.reshape(width, 16).T
    return np.tile(blk, (8, 1))


def _host_prep(feat, src0, dst0, src1, dst1, W_ih, W_hh, b_ih, b_hh):
    deg0 = np.bincount(dst0, minlength=N_NODES)
    deg1 = np.bincount(dst1, minlength=N_NODES)
    w = deg0 + deg1

    # snake-assign nodes (sorted by degree desc) into 392 tiles of <=128
    n_tiles_g = NCORES * TILES
    order = np.argsort(-w, kind="stable")
    tile_of_node = np.empty(N_NODES, np.int32)
    pos_in_tile = np.empty(N_NODES, np.int32)
    tcnt = np.zeros(n_tiles_g, np.int32)
    idx = 0
    fwd = True
    while idx < N_NODES:
        rng = range(n_tiles_g) if fwd else range(n_tiles_g - 1, -1, -1)
        for t in rng:
            if idx >= N_NODES:
                break
            if tcnt[t] < P:
                tile_of_node[order[idx]] = t
                pos_in_tile[order[idx]] = tcnt[t]
                tcnt[t] += 1
                idx += 1
        fwd = not fwd

    # balance tiles over cores by weight
    tile_w = np.zeros(n_tiles_g, np.int64)
    np.add.at(tile_w, tile_of_node, w)
    torder = np.argsort(-tile_w, kind="stable")
    core_of_tile = np.empty(n_tiles_g, np.int32)
    tl_of_tile = np.empty(n_tiles_g, np.int32)
    k = 0
    fwd = True
    for rnd in range(TILES):
        cr = range(NCORES) if fwd else range(NCORES - 1, -1, -1)
        for c in cr:
            core_of_tile[torder[k]] = c
            tl_of_tile[torder[k]] = rnd
            k += 1
        fwd = not fwd

    core_of_node = core_of_tile[tile_of_node]
    slot_of_node = tl_of_tile[tile_of_node] * P + pos_in_tile

    # rebalance nodes within each tile round across the 8 cores so the
    # per-(tile, etype, half) cell counts (whose max-over-cores sets the
    # padded cell size) are as even as possible
    d4 = np.zeros((N_NODES, 4), np.int64)       # (etype, half) edge counts
    for j, (s_, t_) in enumerate([(src0, dst0), (src1, dst1)]):
        for hh in range(2):
            mm = (s_ >= HALF) == (hh == 1)
            np.add.at(d4[:, j * 2 + hh], t_[mm], 1)
    rnd_of_node = tl_of_tile[tile_of_node]
    for r in range(TILES):
        nodes = np.where(rnd_of_node == r)[0]
        nodes = nodes[np.argsort(-w[nodes], kind="stable")]
        cnt8 = np.zeros((NCORES, 4), np.int64)
        fill = np.zeros(NCORES, np.int64)
        wsum = np.zeros(NCORES, np.int64)
        for n in nodes:
            best, bcost = -1, None
            mx = cnt8.max(axis=0)
            for c in range(NCORES):
                if fill[c] >= P:
                    continue
                inc = np.maximum(cnt8[c] + d4[n] - mx, 0).sum()
                cost = (inc, wsum[c])
                if bcost is None or cost < bcost:
                    best, bcost = c, cost
            cnt8[best] += d4[n]
            wsum[best] += w[n]
            core_of_node[n] = best
            slot_of_node[n] = r * P + fill[best]
            fill[best] += 1

    node_of_slot = -np.ones((NCORES, SLOTS), np.int64)
    node_of_slot[core_of_node, slot_of_node] = np.arange(N_NODES)

    # per-node scales: a_e = 1/max(deg_e,1) / max(has0+has1,1)
    has0 = (deg0 > 0).astype(np.float32)
    has1 = (deg1 > 0).astype(np.float32)
    invc = 1.0 / np.maximum(has0 + has1, 1.0)
    a0 = (invc / np.maximum(deg0, 1.0)).astype(np.float32)
    a1 = (invc / np.maximum(deg1, 1.0)).astype(np.float32)

    # edge table
    src_all = np.concatenate([src0, src1])
    dst_all = np.concatenate([dst0, dst1])
    et_all = np.concatenate([np.zeros(len(src0), np.int64),
                             np.ones(len(src1), np.int64)])
    c_all = core_of_node[dst_all]
    s_all = slot_of_node[dst_all]
    t_all = s_all // P
    sp_all = s_all % P
    h_all = (src_all >= HALF).astype(np.int64)
    srel_all = src_all - HALF * h_all

    # cell counts [core, tile, etype, half]
    cell_id = ((t_all * 2 + et_all) * 2 + h_all)
    cnt = np.zeros((NCORES, TILES * 4), np.int64)
    np.add.at(cnt, (c_all, cell_id), 1)
    CS = cnt.max(axis=0)                     # [TILES*4]
    CS = np.maximum((CS + CELL_GRAN - 1) // CELL_GRAN * CELL_GRAN, CELL_GRAN)
    CS = CS.reshape(TILES, 2, 2)             # [tile, etype, half]

    # stream layout per half: cells in (tile, etype) order
    stream_len = [0, 0]
    cell_off = np.zeros((TILES, 2, 2), np.int64)
    for t in range(TILES):
        for e in range(2):
            for h in range(2):
                cell_off[t, e, h] = stream_len[h]
                stream_len[h] += CS[t, e, h]
    # final window of each stream is short: only round up to a chunk (128)
    LA = (stream_len[0] + P - 1) // P * P
    LB = (stream_len[1] + P - 1) // P * P
    stream_len = [LA, LB]
    # gather size of window wi of half h
    nwin = [(LA + WIN - 1) // WIN, (LB + WIN - 1) // WIN]
    win_nidx = {}
    for h in range(2):
        for wi in range(nwin[h]):
            win_nidx[(h, wi)] = min(WIN, stream_len[h] - wi * WIN)

    # piece table (identical across cores): walk each half-stream
    # piece = (half, window, chunk_in_win, tile, etype, pos_lo, pos_hi)
    pieces = []            # in per-half stream order
    for h in range(2):
        bounds = []        # (stream_pos_end, tile, etype)
        for t in range(TILES):
            for e in range(2):
                bounds.append((cell_off[t, e, h] + CS[t, e, h], t, e))
        L = stream_len[h]
        bounds[-1] = (L, bounds[-1][1], bounds[-1][2])
        ci = 0
        pos = 0
        while pos < L:
            chunk_end = pos - pos % P + P
            cell_end, t, e = bounds[ci]
            hi = min(chunk_end, cell_end)
            pieces.append((h, pos // WIN, (pos % WIN) // P, t, e, pos, hi))
            if hi == cell_end and ci + 1 < len(bounds):
                ci += 1
            pos = hi
    NPIECES = len(pieces)

    # program-order interleave of windows: merge by tile progress so the two
    # streams stay within ~1 window of each other in tile space (bounds the
    # number of live PSUM tiles)
    first_tile = {}
    for (h, wi, ck, t, e, lo, hi) in pieces:
        if (h, wi) not in first_tile:
            first_tile[(h, wi)] = t
    win_order = sorted(
        [(h, w) for h in range(2) for w in range(nwin[h])],
        key=lambda hw: (first_tile.get(hw, TILES), hw[1], hw[0]),
    )

    # pieces grouped by (half, window)
    pieces_by_win = {}
    for pi, pc in enumerate(pieces):
        pieces_by_win.setdefault((pc[0], pc[1]), []).append(pi)

    # start/stop flags per (tile, etype) in program order
    order_of_win = {hw: i for i, hw in enumerate(win_order)}
    first_piece = {}
    last_piece = {}
    for pi, (h, wi, ck, t, e, lo, hi) in enumerate(pieces):
        key = (t, e)
        rank = (order_of_win[(h, wi)], pi)
        if key not in first_piece or rank < first_piece[key][0]:
            first_piece[key] = (rank, pi)
        if key not in last_piece or rank > last_piece[key][0]:
            last_piece[key] = (rank, pi)
    start_of = {v[1] for v in first_piece.values()}
    stop_of = {v[1]: k for k, v in last_piece.items()}

    # per-core stream fill
    per_core = []
    for c in range(NCORES):
        m = c_all == c
        key = (h_all[m], t_all[m], et_all[m], sp_all[m])
        o = np.lexsort((key[3], key[2], key[1], key[0]))
        hs, ts, es, sps, srs = (h_all[m][o], t_all[m][o], et_all[m][o],
                                sp_all[m][o], srel_all[m][o])
        sidx = [np.zeros(LA, np.uint16), np.zeros(LB, np.uint16)]
        sdoff = [np.full(LA, 255.0, np.float32), np.full(LB, 255.0, np.float32)]
        # fill cells (edges sorted by (h, t, e, slot))
        cellk = (hs * TILES + ts) * 2 + es
        ccnt = np.bincount(cellk, minlength=TILES * 4)
        cbnd = np.concatenate([[0], np.cumsum(ccnt)])
        for h in range(2):
            for t in range(TILES):
                for e in range(2):
                    kk = (h * TILES + t) * 2 + e
                    lo2, hi2 = cbnd[kk], cbnd[kk + 1]
                    n = hi2 - lo2
                    off = cell_off[t, e, h]
                    assert n <= CS[t, e, h], (c, t, e, h, n, CS[t, e, h])
                    sidx[h][off:off + n] = srs[lo2:hi2]
                    sdoff[h][off:off + n] = sps[lo2:hi2]
        # gidx: wrap16 per window, concatenated in window ISSUE order so the
        # k-th slice of the gidx DMA covers the k-th issued gathers
        gw = [_wrap16(sidx[h][w * WIN:(w + 1) * WIN]) for (h, w) in win_order]
        gidx16 = np.concatenate(gw, axis=1).view(np.int16).copy()
        # doffP [128, NPIECES]
        doffP = np.full((P, NPIECES), 255.0, np.float32)
        for pi, (h, wi, ck, t, e, lo, hi) in enumerate(pieces):
            v = sdoff[h][lo:hi]
            doffP[lo % P:(lo % P) + (hi - lo), pi] = v
        # local node features, transposed, bf16
        sl = node_of_slot[c]
        floc = np.zeros((SLOTS, H), np.float32)
        floc[sl >= 0] = feat[sl[sl >= 0]]
        # per-slot scale tables, replicated across partitions
        a0s = np.zeros(SLOTS, np.float32)
        a1s = np.zeros(SLOTS, np.float32)
        a0s[sl >= 0] = a0[sl[sl >= 0]]
        a1s[sl >= 0] = a1[sl[sl >= 0]]
        a0rep = np.tile(a0s[None, :], (P, 1)).astype(bfloat16)
        a1rep = np.tile(a1s[None, :], (P, 1)).astype(bfloat16)
        per_core.append(dict(
            gidx16=gidx16, doffP=doffP.astype(bfloat16),
            a0rep=a0rep, a1rep=a1rep,
            featloc=floc.T.astype(bfloat16).copy()))

    featA = feat[:HALF].astype(bfloat16)
    featB = np.zeros((HALF, H), np.float32)
    featB[: N_NODES - HALF] = feat[HALF:]
    featB = featB.astype(bfloat16)

    wih = W_ih.T.astype(bfloat16).copy()              # [128, 256]
    whh = W_hh.T.astype(bfloat16).copy()              # [64, 256]
    bt = (b_ih + b_hh).astype(np.float32)
    biasT = np.stack([bt[:128], bt[128:]], axis=1).copy()  # [128, 2]
    iota = np.tile(np.arange(P, dtype=np.float32)[None, :], (P, 1)).astype(bfloat16)

    shared = dict(featA=featA, featB=featB, wih=wih, whh=whh, biasT=biasT,
                  iota=iota)
    meta = dict(pieces=pieces, pieces_by_win=pieces_by_win,
                win_order=win_order, nwin=nwin, NPIECES=NPIECES,
                start_of=start_of, stop_of=stop_of, win_nidx=win_nidx,
                GW=(nwin[0] + nwin[1]) * (WIN // 16))
    return per_core, shared, node_of_slot, meta


_WS = [0]


def _split_multi_waits(nc, mybir, max_waits=1):
    for fn in nc.m.functions:
        for bb in fn.blocks:
            new = []
            for ins in bb.instructions:
                si = ins.sync_info
                if si is not None and len(si.on_wait) > max_waits:
                    waits = list(si.on_wait)
                    for w in waits[:-max_waits]:
                        _WS[0] += 1
                        nop = mybir.InstNoOp(
                            name=f"I-waitsplit-{_WS[0]}", ins=[], outs=[]
                        )
                        nop.engine = ins.engine
                        nop.sync_info = mybir.SyncInfo(on_wait=[w], on_update=[])
                        new.append(nop)
                    si.on_wait = waits[-max_waits:]
                new.append(ins)
            bb.instructions[:] = new


def _build_nc(meta):
    from concourse import bass, mybir, tile, library_config

    f32, bf16, i16 = mybir.dt.float32, mybir.dt.bfloat16, mybir.dt.int16
    pieces = meta["pieces"]
    pieces_by_win = meta["pieces_by_win"]
    win_order = meta["win_order"]
    nwin = meta["nwin"]
    win_nidx = meta["win_nidx"]
    NPIECES = meta["NPIECES"]
    start_of = meta["start_of"]
    stop_of = meta["stop_of"]
    GW = meta["GW"]

    nc = bass.Bass(num_swdge_queues=4)
    featA_d = nc.declare_dram_parameter("featA", [HALF, H], bf16, isOutput=False)
    featB_d = nc.declare_dram_parameter("featB", [HALF, H], bf16, isOutput=False)
    gidx_d = nc.declare_dram_parameter("gidx16", [P, GW], i16, isOutput=False)
    doff_d = nc.declare_dram_parameter("doffP", [P, NPIECES], bf16, isOutput=False)
    a0_d = nc.declare_dram_parameter("a0rep", [P, SLOTS], bf16, isOutput=False)
    a1_d = nc.declare_dram_parameter("a1rep", [P, SLOTS], bf16, isOutput=False)
    wih_d = nc.declare_dram_parameter("wih", [P, 256], bf16, isOutput=False)
    whh_d = nc.declare_dram_parameter("whh", [64, 256], bf16, isOutput=False)
    bias_d = nc.declare_dram_parameter("biasT", [P, 2], f32, isOutput=False)
    iota_d = nc.declare_dram_parameter("iota", [P, P], bf16, isOutput=False)
    floc_d = nc.declare_dram_parameter("featloc", [P, SLOTS], bf16, isOutput=False)
    outT_d = nc.declare_dram_parameter("outT", [P, SLOTS], f32, isOutput=True)

    # window index base (into gidx cols) per (half, wi): issue order
    gidx_col = {}
    col = 0
    for hw in win_order:
        gidx_col[hw] = col
        col += WIN // 16

    with tile.TileContext(nc) as tc:
        with (
            tc.tile_pool(name="const", bufs=1) as cp,
            tc.tile_pool(name="gba", bufs=10) as gba,
            tc.tile_pool(name="gbb", bufs=10) as gbb,
            tc.tile_pool(name="hot", bufs=6) as hp,
            tc.tile_pool(name="ep", bufs=2) as ep,
            tc.tile_pool(name="psm", bufs=2, space="PSUM") as psm,
            tc.tile_pool(name="psg", bufs=1, space="PSUM") as psgp,
        ):
            nc.gpsimd.load_library(library_config.mlp)
            niregs = {n: nc.gpsimd.to_reg(n) for n in sorted(set(win_nidx.values()))}
            # gidx in separate slice tiles so the first gathers start
            # almost immediately (each gather depends only on its slice)
            NSL = 8
            sl = ((GW + NSL - 1) // NSL + 63) // 64 * 64
            gidx_sl = []
            for s in range(0, GW, sl):
                e_ = min(s + sl, GW)
                g_t = cp.tile([P, e_ - s], i16, name=f"gidx{s}")
                gidx_sl.append((s, e_, g_t))
                nc.scalar.dma_start(out=g_t[:], in_=gidx_d[:, s:e_])
                if s == 0:
                    doff = cp.tile([P, NPIECES], bf16)
                    nc.sync.dma_start(out=doff[:], in_=doff_d[:])
                    iota = cp.tile([P, P], bf16)
                    nc.sync.dma_start(out=iota[:], in_=iota_d[:])
                    # warm the scalar HWDGE queue ordering: nothing else
                    # queues there, so slice DMAs complete early

            def gidx_ap(c0, c1):
                for s, e_, g_t in gidx_sl:
                    if c0 >= s and c1 <= e_:
                        return g_t[:, c0 - s:c1 - s]
                raise AssertionError((c0, c1))
            a0rep = cp.tile([P, SLOTS], bf16)
            nc.sync.dma_start(out=a0rep[:], in_=a0_d[:])
            a1rep = cp.tile([P, SLOTS], bf16)
            nc.sync.dma_start(out=a1rep[:], in_=a1_d[:])
            wih = cp.tile([P, 256], bf16)
            nc.sync.dma_start(out=wih[:], in_=wih_d[:])
            whh = cp.tile([64, 256], bf16)
            nc.sync.dma_start(out=whh[:], in_=whh_d[:])
            bias = cp.tile([P, 2], f32)
            nc.sync.dma_start(out=bias[:], in_=bias_d[:])
            featloc = cp.tile([P, SLOTS], bf16)
            nc.sync.dma_start(out=featloc[:], in_=floc_d[:])

            gb_tiles = {}     # (half, wi) -> tile
            hot_tiles = {}    # (half, wi) -> (tile, piece_lo)
            pm_tiles = {}     # (tile, etype) -> psum tile

            qctr = [0]

            def issue_gather(h, wi):
                pool = gba if h == 0 else gbb
                gb = pool.tile([P, WIN // P, P], bf16, tag="gb")
                gb_tiles[(h, wi)] = gb
                nn = win_nidx[(h, wi)]
                nc.gpsimd.dma_gather(
                    out_ap=gb[:, 0:nn // P, :],
                    in_ap=(featA_d if h == 0 else featB_d)[:],
                    idxs_ap=gidx_ap(gidx_col[(h, wi)],
                                    gidx_col[(h, wi)] + nn // 16),
                    num_idxs=nn,
                    num_idxs_reg=niregs[nn],
                    elem_size=H,
                    queue_num=qctr[0] % 4,
                )
                qctr[0] += 1

            PREFETCH = 8
            for k in range(min(PREFETCH, len(win_order))):
                issue_gather(*win_order[k])

            done_tiles = set()
            # count remaining stop flags per tile to trigger epilogue
            stops_needed = {}
            for pi, te in stop_of.items():
                stops_needed.setdefault(te[0], set()).add(te[1])

            def lstm_tile(tl, rstb):
                pg = []
                for half in range(2):
                    g_ps = psgp.tile([P, P], f32, tag=f"pg{half}")
                    nc.tensor.matmul(
                        out=g_ps[:], lhsT=wih[:, half * P:(half + 1) * P],
                        rhs=featloc[:, tl * P:(tl + 1) * P],
                        start=True, stop=False,
                    )
                    nc.tensor.matmul(
                        out=g_ps[:], lhsT=whh[:, half * P:(half + 1) * P],
                        rhs=rstb[0:64, :], start=False, stop=True,
                    )
                    pg.append(g_ps)
                sif = ep.tile([P, P], f32, tag="sif")
                nc.scalar.activation(
                    out=sif[:], in_=pg[0][:],
                    func=mybir.ActivationFunctionType.Sigmoid, bias=bias[:, 0:1],
                )
                sog = ep.tile([P, P], f32, tag="sog")
                nc.scalar.activation(
                    out=sog[0:64, :], in_=pg[1][0:64, :],
                    func=mybir.ActivationFunctionType.Tanh, bias=bias[0:64, 1:2],
                )
                nc.scalar.activation(
                    out=sog[64:128, :], in_=pg[1][64:128, :],
                    func=mybir.ActivationFunctionType.Sigmoid, bias=bias[64:128, 1:2],
                )
                outsb = ep.tile([P, P], f32, tag="outsb")
                t2 = ep.tile([64, P], f32, tag="t2")
                tt = ep.tile([P, P], f32, tag="tt")
                nc.vector.tensor_tensor(
                    out=t2[:], in0=sif[0:64, :], in1=sog[0:64, :],
                    op=mybir.AluOpType.mult,
                )
                nc.scalar.activation(
                    out=tt[64:128, :], in_=t2[:],
                    func=mybir.ActivationFunctionType.Copy,
                )
                nc.vector.tensor_tensor(
                    out=outsb[64:128, :], in0=sif[64:128, :], in1=rstb[64:128, :],
                    op=mybir.AluOpType.mult,
                )
                nc.vector.tensor_tensor(
                    out=outsb[64:128, :], in0=outsb[64:128, :], in1=tt[64:128, :],
                    op=mybir.AluOpType.add,
                )
                nc.scalar.activation(
                    out=tt[64:128, :], in_=outsb[64:128, :],
                    func=mybir.ActivationFunctionType.Tanh,
                )
                nc.vector.tensor_tensor(
                    out=tt[64:128, :], in0=sog[64:128, :], in1=tt[64:128, :],
                    op=mybir.AluOpType.mult,
                )
                nc.scalar.activation(
                    out=outsb[0:64, :], in_=tt[64:128, :],
                    func=mybir.ActivationFunctionType.Copy,
                )
                nc.sync.dma_start(
                    out=outT_d[:, tl * P:(tl + 1) * P], in_=outsb[:]
                )

            for k, (h, wi) in enumerate(win_order):
                if k + PREFETCH < len(win_order):
                    issue_gather(*win_order[k + PREFETCH])
                plist = pieces_by_win.get((h, wi), [])
                if not plist:
                    continue
                p_lo = plist[0]
                npz = len(plist)
                # one is_equal builds all hots of this window
                hot = hp.tile([P, npz, P], bf16, tag="hot")
                nc.vector.tensor_tensor(
                    out=hot[:],
                    in0=doff[:, p_lo:p_lo + npz].to_broadcast([P, npz, P]),
                    in1=iota[:, None, :].to_broadcast([P, npz, P]),
                    op=mybir.AluOpType.is_equal,
                )
                gb = gb_tiles.pop((h, wi))
                for j, pi in enumerate(plist):
                    ph, pwi, ck, t, e, lo, hi = pieces[pi]
                    key = (t, e)
                    if pi in start_of:
                        pm_tiles[key] = psm.tile(
                            [P, P], f32, tag=f"pm{e}", name=f"pm{e}_{t}")
                    nc.tensor.matmul(
                        out=pm_tiles[key][:],
                        lhsT=gb[:, ck, :],
                        rhs=hot[:, j, :],
                        start=(pi in start_of),
                        stop=(pi in stop_of),
                    )
                    if pi in stop_of:
                        te = stop_of[pi]
                        tl = te[0]
                        stops_needed[tl].discard(te[1])
                        if not stops_needed[tl]:
                            # epilogue: rstb = pm0*a0 + pm1*a1 (bf16)
                            pm0 = pm_tiles.pop((tl, 0))
                            pm1 = pm_tiles.pop((tl, 1))
                            tta = ep.tile([P, P], f32, tag="tta")
                            nc.vector.tensor_tensor(
                                out=tta[:], in0=pm0[:],
                                in1=a0rep[:, tl * P:(tl + 1) * P],
                                op=mybir.AluOpType.mult,
                            )
                            ttb = ep.tile([P, P], f32, tag="ttb")
                            nc.vector.tensor_tensor(
                                out=ttb[:], in0=pm1[:],
                                in1=a1rep[:, tl * P:(tl + 1) * P],
                                op=mybir.AluOpType.mult,
                            )
                            rstb = ep.tile([P, P], bf16, tag="rstb")
                            nc.vector.tensor_tensor(
                                out=rstb[:], in0=tta[:], in1=ttb[:],
                                op=mybir.AluOpType.add,
                            )
                            lstm_tile(tl, rstb)

    from concourse import mybir as _mb
    _mb.codegen_inst_isa_subclasses(nc)
    _split_multi_waits(nc, mybir)
    return nc


def kernel(feat, src0, dst0, src1, dst1, W_ih, W_hh, b_ih, b_hh):
    global LAST_EXEC_NS
    feat = np.asarray(feat, np.float32)
    src0 = np.asarray(src0, np.int64); dst0 = np.asarray(dst0, np.int64)
    src1 = np.asarray(src1, np.int64); dst1 = np.asarray(dst1, np.int64)
    per_core, shared, node_of_slot, meta = _host_prep(
        feat, src0, dst0, src1, dst1,
        np.asarray(W_ih, np.float32), np.asarray(W_hh, np.float32),
        np.asarray(b_ih, np.float32), np.asarray(b_hh, np.float32),
    )
    nc = _build_nc(meta)
    in_maps = [{**shared, **pc} for pc in per_core]
    from concourse.bass_utils import run_bass_kernel_spmd
    if TRACE:
        import shutil
        shutil.rmtree("/tmp/trace_out2", ignore_errors=True)
    res = run_bass_kernel_spmd(
        nc, in_maps, list(range(NCORES)), trace=TRACE,
        tmpdir="/tmp/trace_out2" if TRACE else None,
    )
    LAST_EXEC_NS = res.exec_time_ns
    out = np.zeros((N_NODES, H), np.float32)
    for c in range(NCORES):
        oT = res.results[c]["outT"]          # [128, SLOTS]
        valid = node_of_slot[c] >= 0
        nodes = node_of_slot[c][valid]
        out[nodes] = oT.T[valid]
    return out
